# revision 27
# baseline (speedup 1.0000x reference)
"""Trainium2 Bass kernel for CDMamba ModifiedSRCMLayer (self-contained).

Sharding: 8 cores; core k handles batch k//2 and mamba group-pair k%2
(groups {0,1} or {2,3}). Group outputs are exchanged with a paired
AllGather; the post-stage (gate blend + output projection) is computed
redundantly on both cores of a pair and the host reads even cores.

v2: all-bf16 datapath. Phase A/C in bf16 packed matmuls, grouped into
activation-table passes (Sqrt | Sigmoid | Silu). Phase B front-end uses
AF.Silu / AF.Softplus directly; the j-loop broadcasts B/C with PE
matmuls, copies PSUM->SBUF bf16 on the ACT engine, and runs dBu/prod as
[128,2048] pure-bf16 DVE tensor_tensor ops (16-bit 2x rate) plus the
DVE tensor_tensor_scan. The s-reduction accumulates in PSUM via matmul.
"""
import sys
import numpy as np

for _p in ("/opt/trn_rl_repo",):
    if _p not in sys.path:
        sys.path.append(_p)

import concourse.bass as bass
import concourse.mybir as mybir
from concourse.bacc import Bacc
from concourse.tile import TileContext

# Model dims (hardcoded per the problem spec)
B, C, H, W = 4, 128, 64, 64
L = H * W                      # 4096
G, DM = 4, 32
DI, DS, DC = 64, 16, 4
DTR = 2
OUT = 128
EPS = 1e-5

NCORE = 8
LC = 512
NCH = L // LC                  # 8
LH = L // 2                    # 2048
NCC = LH // LC                 # 4
NJ = DS // 2                   # 8 j-tiles (2 s-values per tile)
F32 = mybir.dt.float32
BF = mybir.dt.bfloat16
AF = mybir.ActivationFunctionType
ALU = mybir.AluOpType


def _build_nc():
    nc = Bacc(num_devices=NCORE)

    def inp(name, shape, dt=BF):
        return nc.dram_tensor(name, list(shape), dt, kind="ExternalInput")

    xpad = inp("xpad", (C, 66 * 66))
    pe_b = inp("pe_b", (C, L))
    w9 = inp("w9", (C, 9 * 128))
    mred1 = inp("mred1", (128, 1))
    onesr = inp("onesr", (1, 128))
    gateWT = inp("gateWT", (128, 128))
    gateb = inp("gateb", (128, 1), F32)
    winTu = inp("winTu", (C, 128))       # u for both local groups
    winTz = inp("winTz", (C, 128))
    zb = inp("zb", (128, 1), F32)        # ln-affine fold: Win_z @ ln_b
    ub_neg = inp("ub_neg", (128, 3))     # -Win_u @ ln_b (conv halo)
    conv4T = inp("conv4T", (2, 2, DC, DI, 128))
    convb = inp("convb", (2, 2, 128, 1), F32)
    dtWT = inp("dtWT", (2, 2, DI, 128))
    dtb = inp("dtb", (2, 2, 128, 1), F32)
    xprojJ = inp("xprojJ", (2, 2, NJ, 2, DI, 128))  # replicated B/C weights
    A_col = inp("A_col", (2, 2, 128, NJ), F32)
    dsk = inp("dsk", (2, 2, 128, 1), F32)
    mredM = inp("mredM", (128, DI))
    woutT = inp("woutT", (128, 2 * DM))
    projT = inp("projT", (128, 128))
    projb = inp("projb", (128, 1), F32)

    ym_loc = nc.dram_tensor("ym_loc", [2 * DM, L], BF)
    ym_all = nc.dram_tensor("ym_all", [C, L], BF)
    outp = nc.dram_tensor("outp", [OUT, L], F32, kind="ExternalOutput")

    with TileContext(nc) as tc:
        with (
            tc.tile_pool(name="const", bufs=1) as cp,
            tc.tile_pool(name="big", bufs=1) as bp,
            tc.tile_pool(name="hpool", bufs=2) as hp,
            tc.tile_pool(name="psP", bufs=1, space="PSUM") as psP,
        ):
            # ---- constants to SBUF ----
            def c_load(ap_dram, shape, nm, dt=BF):
                t = cp.tile(list(shape), dt, name=nm, tag=nm)
                nc.sync.dma_start(t[:], ap_dram)
                return t

            w9_sb = c_load(w9[:], (C, 9 * 128), "w9sb")
            mred1_sb = c_load(mred1[:], (128, 1), "mred1sb")
            onesr_sb = c_load(onesr[:], (1, 128), "onesrsb")
            gateWT_sb = c_load(gateWT[:], (128, 128), "gateWTsb")
            gateb_sb = c_load(gateb[:], (128, 1), "gatebsb", F32)
            winTu_sb = c_load(winTu[:], (C, 128), "winTusb")
            winTz_sb = c_load(winTz[:], (C, 128), "winTzsb")
            zb_sb = c_load(zb[:], (128, 1), "zbsb", F32)
            ubn_sb = c_load(ub_neg[:], (128, 3), "ubnsb")
            mredM_sb = c_load(mredM[:], (128, DI), "mredMsb")
            woutT_sb = c_load(woutT[:], (128, 2 * DM), "woutTsb")
            projT_sb = c_load(projT[:], (128, 128), "projTsb")
            projb_sb = c_load(projb[:], (128, 1), "projbsb", F32)

            conv4T_sb = cp.tile([128, 16 * 128], BF)
            dtWT_sb = cp.tile([DI, 4 * 128], BF)
            xprojJ_sb = cp.tile([DI, 4 * NJ * 2 * 128], BF)
            acol_sb = cp.tile([128, 4 * NJ], F32)
            convb_sb = cp.tile([128, 4], F32)
            dtb_sb = cp.tile([128, 4], F32)
            dsk_sb = cp.tile([128, 4], F32)
            eps_sb = cp.tile([1, 1], F32)
            nc.vector.memset(eps_sb[:], EPS)
            for gl in range(2):
                for dr in range(2):
                    i4 = gl * 2 + dr
                    for k in range(DC):
                        for hh in range(2):
                            nc.sync.dma_start(
                                conv4T_sb[hh * 64:(hh + 1) * 64,
                                          (i4 * 4 + k) * 128:(i4 * 4 + k + 1) * 128],
                                conv4T[gl, dr, k])
                    nc.sync.dma_start(dtWT_sb[:, i4 * 128:(i4 + 1) * 128],
                                      dtWT[gl, dr])
                    for j in range(NJ):
                        for sd_ in range(2):
                            o = ((i4 * NJ + j) * 2 + sd_) * 128
                            nc.sync.dma_start(xprojJ_sb[:, o:o + 128],
                                              xprojJ[gl, dr, j, sd_])
                    nc.sync.dma_start(acol_sb[:, i4 * NJ:(i4 + 1) * NJ],
                                      A_col[gl, dr])
                    nc.sync.dma_start(convb_sb[:, i4:i4 + 1], convb[gl, dr])
                    nc.sync.dma_start(dtb_sb[:, i4:i4 + 1], dtb[gl, dr])
                    nc.sync.dma_start(dsk_sb[:, i4:i4 + 1], dsk[gl, dr])

            # ---- persistent tiles ----
            xs = bp.tile([C, L], BF)
            gate = bp.tile([C, L], BF)
            u_pad = bp.tile([C, L + 6], BF)
            zs = bp.tile([C, L], BF)
            yfb = bp.tile([C, L], BF)

            # halo = -Win_u@ln_b so the folded-LN conv matches zero-padded ref
            nc.vector.tensor_copy(u_pad[:, 0:3], ubn_sb[:])
            nc.vector.tensor_copy(u_pad[:, L + 3:L + 6], ubn_sb[:])

            # ---- Phase A ----
            with tc.tile_pool(name="pA", bufs=2) as pA:
                xpad_sb = pA.tile([C, 66 * 66], BF, bufs=1)
                nc.sync.dma_start(xpad_sb[:], xpad[:])
                xpad3 = xpad_sb[:].rearrange("p (r q) -> p r q", q=66)
                xnc = pA.tile([C, L], BF, bufs=1)   # centered/normed (LN affine folded)
                xcf = pA.tile([C, L], BF, bufs=1)   # centered
                # pass0: pos-enc conv, dense PE burst (no tables)
                for c in range(NCH):
                    cs = slice(c * LC, (c + 1) * LC)
                    pa = psP.tile([128, 8, 64], F32, tag="gen", bufs=2)
                    for tap in range(9):
                        dy, dx = tap // 3, tap % 3
                        nc.tensor.matmul(
                            pa[:],
                            w9_sb[:, tap * 128:(tap + 1) * 128],
                            xpad3[:, c * 8 + dy:c * 8 + dy + 8, dx:dx + 64],
                            start=(tap == 0), stop=(tap == 8))
                    paf = pa[:].rearrange("p a b -> p (a b)")
                    pe_t = pA.tile([128, LC], BF, tag="pe")
                    nc.sync.dma_start(pe_t[:], pe_b[:, cs])
                    nc.vector.tensor_tensor(xs[:, cs], paf, pe_t[:], op=ALU.add)
                # pass1: LN (tables: Sqrt; Square free)
                for c in range(NCH):
                    cs = slice(c * LC, (c + 1) * LC)
                    mu = psP.tile([1, LC], F32, tag="gen", bufs=2)
                    nc.tensor.matmul(mu[:], mred1_sb[:], xs[:, cs],
                                     start=True, stop=True)
                    mu_sb = pA.tile([1, LC], BF, tag="musb")
                    nc.scalar.copy(mu_sb[:], mu[:])
                    mub = psP.tile([128, LC], F32, tag="gen", bufs=2)
                    nc.tensor.matmul(mub[:], onesr_sb[:], mu_sb[:],
                                     start=True, stop=True)
                    nc.vector.tensor_tensor(xcf[:, cs], xs[:, cs], mub[:],
                                            op=ALU.subtract)
                    xsq = pA.tile([128, LC], BF, tag="xsq")
                    nc.vector.tensor_tensor(xsq[:], xcf[:, cs], xcf[:, cs],
                                            op=ALU.mult)
                    var = psP.tile([1, LC], F32, tag="gen", bufs=2)
                    nc.tensor.matmul(var[:], mred1_sb[:], xsq[:],
                                     start=True, stop=True)
                    sd = pA.tile([1, LC], F32, tag="sd")
                    nc.scalar.activation(sd[:], var[:], AF.Sqrt,
                                         bias=eps_sb[:, 0:1])
                    rstd = pA.tile([1, LC], BF, tag="rstd")
                    with nc.allow_low_precision(reason="bf16 rstd; tol 2e-2"):
                        nc.vector.reciprocal(rstd[:], sd[:])
                    rstdb = psP.tile([128, LC], F32, tag="gen", bufs=2)
                    nc.tensor.matmul(rstdb[:], onesr_sb[:], rstd[:],
                                     start=True, stop=True)
                    nc.vector.tensor_tensor(xnc[:, cs], xcf[:, cs], rstdb[:],
                                            op=ALU.mult)
                # pass2: gate (Sigmoid)
                for c in range(NCH):
                    cs = slice(c * LC, (c + 1) * LC)
                    gps = psP.tile([128, LC], F32, tag="gen", bufs=2)
                    nc.tensor.matmul(gps[:], gateWT_sb[:], xnc[:, cs],
                                     start=True, stop=True)
                    nc.scalar.activation(gate[:, cs], gps[:], AF.Sigmoid,
                                         bias=gateb_sb[:, 0:1])
                # pass3: xz both local groups packed (Silu; Copy free)
                for c in range(NCH):
                    cs = slice(c * LC, (c + 1) * LC)
                    up = psP.tile([128, LC], F32, tag="gen", bufs=2)
                    nc.tensor.matmul(up[:], winTu_sb[:], xnc[:, cs],
                                     start=True, stop=True)
                    nc.scalar.copy(u_pad[:, 3 + c * LC:3 + (c + 1) * LC], up[:])
                    zp = psP.tile([128, LC], F32, tag="gen", bufs=2)
                    nc.tensor.matmul(zp[:], winTz_sb[:], xnc[:, cs],
                                     start=True, stop=True)
                    nc.scalar.activation(zs[:, cs], zp[:], AF.Silu,
                                         bias=zb_sb[:, 0:1])

            # ---- Phase B ----
            # All four (gl,dr) front-ends first (one table set per pass),
            # then the four j-loops (Exp table once).
            with tc.tile_pool(name="pB", bufs=2) as wp:
                uc_all = [bp.tile([128, L], BF, name=f"uc{i4}")
                          for i4 in range(4)]
                dt_all = [bp.tile([128, L], BF, name=f"dtt{i4}")
                          for i4 in range(4)]
                # FE pass1: conv + silu (Silu table)
                for gl in range(2):
                    rows = slice(gl * 64, gl * 64 + 64)
                    for dr in range(2):
                        i4 = gl * 2 + dr
                        uc = uc_all[i4]
                        for cc in range(NCH // 2):
                            ucp = psP.tile([128, 2 * LC], F32, tag="gen", bufs=2)
                            for half in range(2):
                                c = cc * 2 + half
                                for k in range(DC):
                                    off = (c * LC + k) if dr == 0 else (3 + c * LC + k)
                                    nc.tensor.matmul(
                                        ucp[:, half * LC:(half + 1) * LC],
                                        conv4T_sb[rows,
                                                  (i4 * 4 + k) * 128:
                                                  (i4 * 4 + k + 1) * 128],
                                        u_pad[rows, off:off + LC],
                                        start=(k == 0), stop=(k == DC - 1))
                            nc.scalar.activation(
                                uc[:, cc * 2 * LC:(cc + 1) * 2 * LC], ucp[:],
                                AF.Silu, bias=convb_sb[:, i4:i4 + 1])
                # FE pass2: dt matmul + sigmoid (Sigmoid table)
                for i4 in range(4):
                    uc, dt_t = uc_all[i4], dt_all[i4]
                    for cc in range(NCH // 2):
                        c2s = slice(cc * 2 * LC, (cc + 1) * 2 * LC)
                        dtp = psP.tile([128, 2 * LC], F32, tag="gen", bufs=2)
                        for half in range(2):
                            c = cc * 2 + half
                            nc.tensor.matmul(
                                dtp[:, half * LC:(half + 1) * LC],
                                dtWT_sb[:, i4 * 128:(i4 + 1) * 128],
                                uc[0:DI, c * LC:(c + 1) * LC],
                                start=True, stop=True)
                        nc.scalar.activation(dt_t[:, c2s], dtp[:],
                                             AF.Sigmoid,
                                             bias=dtb_sb[:, i4:i4 + 1],
                                             scale=-1.0)
                # FE pass3: dt_t = ln(sigmoid(..)) = -softplus (Ln table)
                for i4 in range(4):
                    nc.scalar.activation(dt_all[i4][:], dt_all[i4][:], AF.Ln)

                # j-loops (Exp table; Copy free)
                for gl in range(2):
                    rows = slice(gl * 64, gl * 64 + 64)
                    for dr in range(2):
                        i4 = gl * 2 + dr
                        uc, dt_t = uc_all[i4], dt_all[i4]
                        dtuc = wp.tile([128, L], BF, tag="dtuc", bufs=2)
                        nc.vector.tensor_tensor(dtuc[:], dt_t[:], uc[:],
                                                op=ALU.mult)
                        horder = (0, 1) if dr == 0 else (1, 0)
                        h_prev = [None] * NJ
                        for hf in horder:
                            hs = slice(hf * LH, (hf + 1) * LH)
                            first = (hf == horder[0])
                            ys = [psP.tile([128, LC], F32, tag=f"ys{q}",
                                           bufs=1, name=f"ys{q}")
                                  for q in range(NCC)]
                            prev_prod = None
                            # rr = exp(2*dt_t) steps the s-pair decay by 2
                            rr = wp.tile([128, LH], BF, tag="rr", bufs=2)
                            nc.scalar.activation(rr[:], dt_t[:, hs], AF.Exp,
                                                 scale=2.0)
                            prev_dA = None

                            def bcast(side, j):
                                # B/C broadcast for s-pair j, direct from uc
                                bb = wp.tile([128, LH], BF,
                                             tag=("bbB" if side == 0 else "bbC"))
                                wsl = ((i4 * NJ + j) * 2 + side) * 128
                                for p2 in range(2):
                                    bps = psP.tile([128, 2 * LC], F32,
                                                   tag="gen", bufs=2)
                                    for half in range(2):
                                        q = p2 * 2 + half
                                        nc.tensor.matmul(
                                            bps[:, half * LC:(half + 1) * LC],
                                            xprojJ_sb[:, wsl:wsl + 128],
                                            uc[0:DI, hf * LH + q * LC:
                                               hf * LH + (q + 1) * LC],
                                            start=True, stop=True)
                                    nc.scalar.copy(
                                        bb[:, p2 * 2 * LC:(p2 + 1) * 2 * LC],
                                        bps[:])
                                return bb

                            for j in range(NJ):
                                dA = wp.tile([128, LH], BF, tag="dA")
                                if j == 0:
                                    nc.scalar.activation(
                                        dA[:], dt_t[:, hs], AF.Exp,
                                        scale=acol_sb[:, i4 * NJ:i4 * NJ + 1])
                                else:
                                    nc.gpsimd.tensor_tensor(dA[:], prev_dA[:],
                                                            rr[:], op=ALU.mult)
                                prev_dA = dA
                                bbB = bcast(0, j)
                                dBu = wp.tile([128, LH], BF, tag="dBu")
                                nc.vector.tensor_tensor(dBu[:], dtuc[:, hs],
                                                        bbB[:], op=ALU.mult)
                                h = hp.tile([128, LH], BF, tag="h")
                                hc = hp.tile([128, 1], BF, tag=f"hc{j}",
                                             name=f"hc{j}")
                                init = 0.0 if first else h_prev[j][:, 0:1]
                                if dr == 0:
                                    nc.vector.tensor_tensor_scan(
                                        h[:], dA[:], dBu[:], init,
                                        op0=ALU.mult, op1=ALU.add)
                                    nc.scalar.copy(hc[:], h[:, LH - 1:LH])
                                else:
                                    nc.vector.tensor_tensor_scan(
                                        h[:, ::-1], dA[:, ::-1], dBu[:, ::-1],
                                        init, op0=ALU.mult, op1=ALU.add)
                                    nc.scalar.copy(hc[:], h[:, 0:1])
                                h_prev[j] = hc
                                bbC = bcast(1, j)
                                prod = wp.tile([128, LH], BF, tag="prod")
                                nc.vector.tensor_tensor(prod[:], h[:], bbC[:],
                                                        op=ALU.mult)
                                if prev_prod is not None:
                                    for q in range(NCC):
                                        nc.tensor.matmul(
                                            ys[q][rows, :], mredM_sb[:, 0:DI],
                                            prev_prod[:, q * LC:(q + 1) * LC],
                                            start=(j - 1 == 0), stop=False)
                                prev_prod = prod
                            for q in range(NCC):
                                nc.tensor.matmul(
                                    ys[q][rows, :], mredM_sb[:, 0:DI],
                                    prev_prod[:, q * LC:(q + 1) * LC],
                                    start=(NJ == 1), stop=True)
                            # epilogue for this half
                            for q in range(NCC):
                                c = hf * NCC + q
                                cs = slice(c * LC, (c + 1) * LC)
                                y1 = wp.tile([128, LC], BF, tag="y1")
                                nc.vector.scalar_tensor_tensor(
                                    y1[rows, :], uc[rows, cs],
                                    dsk_sb[rows, i4:i4 + 1],
                                    ys[q][rows, :], op0=ALU.mult,
                                    op1=ALU.subtract)
                                if dr == 0:
                                    nc.gpsimd.tensor_tensor(yfb[rows, cs],
                                                            y1[rows, :],
                                                            zs[rows, cs],
                                                            op=ALU.mult)
                                else:
                                    y2 = wp.tile([128, LC], BF, tag="y2")
                                    nc.gpsimd.tensor_tensor(y2[rows, :],
                                                            y1[rows, :],
                                                            zs[rows, cs],
                                                            op=ALU.mult)
                                    nc.gpsimd.tensor_tensor(yfb[rows, cs],
                                                            yfb[rows, cs],
                                                            y2[rows, :],
                                                            op=ALU.add)

            # ---- Phase C: Wout, exchange, blend, proj ----
            with tc.tile_pool(name="pC", bufs=2) as wpc:
                for c in range(NCH):
                    cs = slice(c * LC, (c + 1) * LC)
                    ymp = psP.tile([2 * DM, LC], F32, tag="gen", bufs=2)
                    nc.tensor.matmul(ymp[:], woutT_sb[:], yfb[:, cs],
                                     start=True, stop=True)
                    ym_sb = wpc.tile([2 * DM, LC], BF, tag="ymsb")
                    nc.scalar.copy(ym_sb[:], ymp[:])
                    nc.sync.dma_start(ym_loc[:, cs], ym_sb[:])
                nc.gpsimd.collective_compute(
                    "AllGather", ALU.bypass,
                    replica_groups=[[0, 1], [2, 3], [4, 5], [6, 7]],
                    ins=[ym_loc[:]], outs=[ym_all[:]])
                for c in range(NCH):
                    cs = slice(c * LC, (c + 1) * LC)
                    xm_t = wpc.tile([C, LC], BF, tag="xmt")
                    nc.sync.dma_start(xm_t[:], ym_all[:, cs])
                    ta = wpc.tile([128, LC], BF, tag="ta")
                    nc.gpsimd.tensor_tensor(ta[:], xm_t[:], xs[:, cs],
                                            op=ALU.subtract)
                    tb2 = wpc.tile([128, LC], BF, tag="tb")
                    nc.vector.tensor_tensor(tb2[:], gate[:, cs], ta[:],
                                            op=ALU.mult)
                    tc2 = wpc.tile([128, LC], BF, tag="tc")
                    nc.gpsimd.tensor_tensor(tc2[:], xs[:, cs], tb2[:],
                                            op=ALU.add)
                    op_ = psP.tile([128, LC], F32, tag="gen", bufs=2)
                    nc.tensor.matmul(op_[:], projT_sb[:], tc2[:],
                                     start=True, stop=True)
                    osb = wpc.tile([128, LC], F32, tag="osb")
                    nc.scalar.activation(osb[:], op_[:], AF.Identity,
                                         bias=projb_sb[:, 0:1])
                    nc.sync.dma_start(outp[:, cs], osb[:])
    nc.finalize()
    return nc


def _bf(a):
    import concourse.mybir as _mb
    return np.asarray(a).astype(_mb.dt.np(_mb.dt.bfloat16))


def _prep_inputs(inputs):
    """Build the 8 per-core in_maps from full inputs."""
    ii = {k: np.asarray(v, dtype=np.float32) for k, v in inputs.items()}
    x = ii["x"]

    maps_w = []  # weight dicts per group-set gs=0,1
    for gs in range(2):
        w = {}
        w9 = np.zeros((C, 9 * 128), np.float32)
        for tap in range(9):
            dy, dx = tap // 3, tap % 3
            blk = np.zeros((C, 128), np.float32)
            np.fill_diagonal(blk, ii["pos_conv_w"][:, 0, dy, dx])
            if tap == 4:
                blk[np.arange(C), np.arange(C)] += 1.0
            w9[:, tap * 128:(tap + 1) * 128] = blk
        w["w9"] = _bf(w9)
        w["pe_b"] = _bf(np.ascontiguousarray(ii["pos_embed"][0].T)
                        + ii["pos_conv_b"][:, None])
        w["mred1"] = _bf(np.full((128, 1), 1.0 / 128, np.float32))
        w["onesr"] = _bf(np.ones((1, 128), np.float32))
        lng = ii["ln_g"]
        lnb = ii["ln_b"]
        # LN affine folded into consumers: xnc on-device is (x-mu)/sd
        w["gateWT"] = _bf(ii["gate_W"].T * lng[:, None])
        w["gateb"] = np.ascontiguousarray(
            (ii["gate_b"] + ii["gate_W"] @ lnb)[:, None])
        w["projT"] = _bf(ii["proj_W"].T)
        w["projb"] = np.ascontiguousarray(ii["proj_b"][:, None])
        w["mredM"] = _bf(np.tile(np.eye(DI, dtype=np.float32), (2, 1)))
        winTu = np.zeros((C, 128), np.float32)
        winTz = np.zeros((C, 128), np.float32)
        zb = np.zeros((128, 1), np.float32)
        ub = np.zeros((128,), np.float32)
        conv4T = np.zeros((2, 2, DC, DI, 128), np.float32)
        convb = np.zeros((2, 2, 128, 1), np.float32)
        dtWT = np.zeros((2, 2, DI, 128), np.float32)
        dtb = np.zeros((2, 2, 128, 1), np.float32)
        xprojJ = np.zeros((2, 2, NJ, 2, DI, 128), np.float32)
        A_col = np.zeros((2, 2, 128, NJ), np.float32)
        dsk = np.zeros((2, 2, 128, 1), np.float32)
        woutT = np.zeros((128, 2 * DM), np.float32)
        for gl in range(2):
            gg = gs * 2 + gl
            gsl = slice(gg * DM, (gg + 1) * DM)
            Wu = ii["m_Win"][gg, 0:DI, :]        # (DI, DM)
            Wz = ii["m_Win"][gg, DI:2 * DI, :]
            winTu[gsl, gl * DI:(gl + 1) * DI] = (Wu * lng[None, gsl]).T
            winTz[gsl, gl * DI:(gl + 1) * DI] = (Wz * lng[None, gsl]).T
            ub[gl * DI:(gl + 1) * DI] = Wu @ lnb[gsl]
            zb[gl * DI:(gl + 1) * DI, 0] = Wz @ lnb[gsl]
            woutT[gl * 64:(gl + 1) * 64, gl * DM:(gl + 1) * DM] = ii["m_Wout"][gg].T
            for dr in range(2):
                for k in range(DC):
                    wk = ii["conv_w"][gg, dr, :, k if dr == 0 else DC - 1 - k]
                    blk = np.zeros((DI, 128), np.float32)
                    blk[np.arange(DI), np.arange(DI)] = wk
                    blk[np.arange(DI), 64 + np.arange(DI)] = wk
                    conv4T[gl, dr, k] = blk
                convb[gl, dr, :, 0] = np.tile(
                    ii["conv_b"][gg, dr]
                    + ii["conv_w"][gg, dr].sum(-1) * ub[gl * DI:(gl + 1) * DI], 2)
                M2 = ii["dt_W"][gg, dr] @ ii["xproj_W"][gg, dr][0:DTR, :]
                dtWT[gl, dr] = np.concatenate([M2.T, M2.T], axis=1)
                dtb[gl, dr, :, 0] = -np.tile(ii["dt_b"][gg, dr], 2)
                Wb = ii["xproj_W"][gg, dr][DTR:DTR + DS, :]        # (DS, DI)
                Wc = ii["xproj_W"][gg, dr][DTR + DS:DTR + 2 * DS, :]
                p = np.arange(128)
                for j in range(NJ):
                    xprojJ[gl, dr, j, 0] = Wb[2 * j + p[None, :] // 64,
                                              np.arange(DI)[:, None]]
                    xprojJ[gl, dr, j, 1] = Wc[2 * j + p[None, :] // 64,
                                              np.arange(DI)[:, None]]
                A = np.exp(ii["A_log"][gg, dr])  # (DI, DS); dt_t is -dt
                for j in range(NJ):
                    A_col[gl, dr, :, j] = A[p % 64, 2 * j + p // 64]
                dsk[gl, dr, :, 0] = np.tile(ii["Dskip"][gg, dr], 2)
        w.update(winTu=_bf(winTu), winTz=_bf(winTz), zb=zb,
                 ub_neg=_bf(np.tile(-ub[:, None], (1, 3))),
                 conv4T=_bf(conv4T), convb=convb, dtWT=_bf(dtWT), dtb=dtb,
                 xprojJ=_bf(xprojJ), A_col=A_col, dsk=dsk, woutT=_bf(woutT))
        maps_w.append(w)

    in_maps = []
    for k in range(NCORE):
        b, gs = k // 2, k % 2
        m = dict(maps_w[gs])
        xp = np.zeros((C, 66, 66), np.float32)
        xp[:, 1:65, 1:65] = x[b]
        m["xpad"] = _bf(np.ascontiguousarray(xp.reshape(C, 66 * 66)))
        in_maps.append(m)
    return in_maps


_CACHE = {}


def kernel(**inputs):
    from concourse.bass_utils import run_bass_kernel_spmd
    if "nc" not in _CACHE:
        _CACHE["nc"] = _build_nc()
    nc = _CACHE["nc"]
    in_maps = _prep_inputs(inputs)
    res = run_bass_kernel_spmd(nc, in_maps, list(range(NCORE))).results
    out = np.stack([np.asarray(res[2 * b]["outp"]).reshape(OUT, H, W)
                    for b in range(B)])
    return out.astype(np.float32)


# revision 29
# speedup vs baseline: 1.3670x; 1.3670x over previous
"""Trainium2 Bass kernel for CDMamba ModifiedSRCMLayer (self-contained).

Sharding: 8 cores; core k handles batch k//2 and mamba group-pair k%2
(groups {0,1} or {2,3}). Group outputs are exchanged with a paired
AllGather; the post-stage (gate blend + output projection) is computed
redundantly on both cores of a pair and the host reads even cores.

v2: all-bf16 datapath. Phase A/C in bf16 packed matmuls, grouped into
activation-table passes (Sqrt | Sigmoid | Silu). Phase B front-end uses
AF.Silu / AF.Softplus directly; the j-loop broadcasts B/C with PE
matmuls, copies PSUM->SBUF bf16 on the ACT engine, and runs dBu/prod as
[128,2048] pure-bf16 DVE tensor_tensor ops (16-bit 2x rate) plus the
DVE tensor_tensor_scan. The s-reduction accumulates in PSUM via matmul.
"""
import sys
import numpy as np

for _p in ("/opt/trn_rl_repo",):
    if _p not in sys.path:
        sys.path.append(_p)

import concourse.bass as bass
import concourse.mybir as mybir
from concourse.bacc import Bacc
from concourse.tile import TileContext

# Model dims (hardcoded per the problem spec)
B, C, H, W = 4, 128, 64, 64
L = H * W                      # 4096
G, DM = 4, 32
DI, DS, DC = 64, 16, 4
DTR = 2
OUT = 128
EPS = 1e-5

NCORE = 8
LC = 512
NCH = L // LC                  # 8
LH = L // 2                    # 2048
NCC = LH // LC                 # 4
NJ = DS // 2                   # 8 j-tiles (2 s-values per tile)
F32 = mybir.dt.float32
BF = mybir.dt.bfloat16
AF = mybir.ActivationFunctionType
ALU = mybir.AluOpType


def _build_nc():
    nc = Bacc(num_devices=NCORE)

    def inp(name, shape, dt=BF):
        return nc.dram_tensor(name, list(shape), dt, kind="ExternalInput")

    xpad = inp("xpad", (C, 66 * 66))
    pe_b = inp("pe_b", (C, L))
    w9 = inp("w9", (C, 9 * 128))
    mred1 = inp("mred1", (128, 1))
    onesr = inp("onesr", (1, 128))
    gateWT = inp("gateWT", (128, 128))
    gateb = inp("gateb", (128, 1), F32)
    winTu = inp("winTu", (C, 128))       # u for both local groups
    winTz = inp("winTz", (C, 128))
    zb = inp("zb", (128, 1), F32)        # ln-affine fold: Win_z @ ln_b
    ub_neg = inp("ub_neg", (128, 3))     # -Win_u @ ln_b (conv halo)
    conv4T = inp("conv4T", (2, 2, DC, DI, 128))
    convb = inp("convb", (2, 2, 128, 1), F32)
    dtWT = inp("dtWT", (2, 2, DI, 128))
    dtb = inp("dtb", (2, 2, 128, 1), F32)
    xprojJ = inp("xprojJ", (2, 2, NJ, 2, DI, 128))  # replicated B/C weights
    A_col = inp("A_col", (2, 2, 128, NJ), F32)
    dsk = inp("dsk", (2, 2, 128, 1), F32)
    mredM = inp("mredM", (128, DI))
    woutT = inp("woutT", (128, 2 * DM))
    projT = inp("projT", (128, 128))
    projb = inp("projb", (128, 1), F32)

    ym_loc = nc.dram_tensor("ym_loc", [2 * DM, L], BF)
    ym_all = nc.dram_tensor("ym_all", [C, L], BF)
    outp = nc.dram_tensor("outp", [OUT, L], F32, kind="ExternalOutput")

    with TileContext(nc) as tc:
        with (
            tc.tile_pool(name="const", bufs=1) as cp,
            tc.tile_pool(name="big", bufs=1) as bp,
            tc.tile_pool(name="hpool", bufs=2) as hp,
            tc.tile_pool(name="psP", bufs=1, space="PSUM") as psP,
        ):
            # ---- constants to SBUF ----
            def c_load(ap_dram, shape, nm, dt=BF):
                t = cp.tile(list(shape), dt, name=nm, tag=nm)
                nc.sync.dma_start(t[:], ap_dram)
                return t

            w9_sb = c_load(w9[:], (C, 9 * 128), "w9sb")
            mred1_sb = c_load(mred1[:], (128, 1), "mred1sb")
            onesr_sb = c_load(onesr[:], (1, 128), "onesrsb")
            gateWT_sb = c_load(gateWT[:], (128, 128), "gateWTsb")
            gateb_sb = c_load(gateb[:], (128, 1), "gatebsb", F32)
            winTu_sb = c_load(winTu[:], (C, 128), "winTusb")
            winTz_sb = c_load(winTz[:], (C, 128), "winTzsb")
            zb_sb = c_load(zb[:], (128, 1), "zbsb", F32)
            ubn_sb = c_load(ub_neg[:], (128, 3), "ubnsb")
            mredM_sb = c_load(mredM[:], (128, DI), "mredMsb")
            woutT_sb = c_load(woutT[:], (128, 2 * DM), "woutTsb")
            projT_sb = c_load(projT[:], (128, 128), "projTsb")
            projb_sb = c_load(projb[:], (128, 1), "projbsb", F32)

            conv4T_sb = cp.tile([128, 16 * 128], BF)
            dtWT_sb = cp.tile([DI, 4 * 128], BF)
            xprojJ_sb = cp.tile([DI, 4 * NJ * 2 * 128], BF)
            acol_sb = cp.tile([128, 4 * NJ], F32)
            convb_sb = cp.tile([128, 4], F32)
            dtb_sb = cp.tile([128, 4], F32)
            dsk_sb = cp.tile([128, 4], F32)
            eps_sb = cp.tile([1, 1], F32)
            nc.vector.memset(eps_sb[:], EPS)
            for gl in range(2):
                for dr in range(2):
                    i4 = gl * 2 + dr
                    for k in range(DC):
                        for hh in range(2):
                            nc.sync.dma_start(
                                conv4T_sb[hh * 64:(hh + 1) * 64,
                                          (i4 * 4 + k) * 128:(i4 * 4 + k + 1) * 128],
                                conv4T[gl, dr, k])
                    nc.sync.dma_start(dtWT_sb[:, i4 * 128:(i4 + 1) * 128],
                                      dtWT[gl, dr])
                    for j in range(NJ):
                        for sd_ in range(2):
                            o = ((i4 * NJ + j) * 2 + sd_) * 128
                            nc.sync.dma_start(xprojJ_sb[:, o:o + 128],
                                              xprojJ[gl, dr, j, sd_])
                    nc.sync.dma_start(acol_sb[:, i4 * NJ:(i4 + 1) * NJ],
                                      A_col[gl, dr])
                    nc.sync.dma_start(convb_sb[:, i4:i4 + 1], convb[gl, dr])
                    nc.sync.dma_start(dtb_sb[:, i4:i4 + 1], dtb[gl, dr])
                    nc.sync.dma_start(dsk_sb[:, i4:i4 + 1], dsk[gl, dr])

            # ---- persistent tiles ----
            xs = bp.tile([C, L], BF)
            gate = bp.tile([C, L], BF)
            u_pad = bp.tile([C, L + 6], BF)
            zs = bp.tile([C, L], BF)
            yfb = bp.tile([C, L], BF)

            # halo = -Win_u@ln_b so the folded-LN conv matches zero-padded ref
            nc.vector.tensor_copy(u_pad[:, 0:3], ubn_sb[:])
            nc.vector.tensor_copy(u_pad[:, L + 3:L + 6], ubn_sb[:])

            # ---- Phase A ----
            with tc.tile_pool(name="pA", bufs=2) as pA:
                xpad_sb = pA.tile([C, 66 * 66], BF, bufs=1)
                nc.sync.dma_start(xpad_sb[:], xpad[:])
                xpad3 = xpad_sb[:].rearrange("p (r q) -> p r q", q=66)
                xnc = pA.tile([C, L], BF, bufs=1)   # centered/normed (LN affine folded)
                xcf = pA.tile([C, L], BF, bufs=1)   # centered
                # pass0: pos-enc conv, dense PE burst (no tables)
                for c in range(NCH):
                    cs = slice(c * LC, (c + 1) * LC)
                    pa = psP.tile([128, 8, 64], F32, tag="gen", bufs=2)
                    for tap in range(9):
                        dy, dx = tap // 3, tap % 3
                        nc.tensor.matmul(
                            pa[:],
                            w9_sb[:, tap * 128:(tap + 1) * 128],
                            xpad3[:, c * 8 + dy:c * 8 + dy + 8, dx:dx + 64],
                            start=(tap == 0), stop=(tap == 8))
                    paf = pa[:].rearrange("p a b -> p (a b)")
                    pe_t = pA.tile([128, LC], BF, tag="pe")
                    nc.sync.dma_start(pe_t[:], pe_b[:, cs])
                    nc.vector.tensor_tensor(xs[:, cs], paf, pe_t[:], op=ALU.add)
                # pass1: LN (tables: Sqrt; Square free)
                for c in range(NCH):
                    cs = slice(c * LC, (c + 1) * LC)
                    mu = psP.tile([1, LC], F32, tag="gen", bufs=2)
                    nc.tensor.matmul(mu[:], mred1_sb[:], xs[:, cs],
                                     start=True, stop=True)
                    mu_sb = pA.tile([1, LC], BF, tag="musb")
                    nc.scalar.copy(mu_sb[:], mu[:])
                    mub = psP.tile([128, LC], F32, tag="gen", bufs=2)
                    nc.tensor.matmul(mub[:], onesr_sb[:], mu_sb[:],
                                     start=True, stop=True)
                    nc.vector.tensor_tensor(xcf[:, cs], xs[:, cs], mub[:],
                                            op=ALU.subtract)
                    xsq = pA.tile([128, LC], BF, tag="xsq")
                    nc.vector.tensor_tensor(xsq[:], xcf[:, cs], xcf[:, cs],
                                            op=ALU.mult)
                    var = psP.tile([1, LC], F32, tag="gen", bufs=2)
                    nc.tensor.matmul(var[:], mred1_sb[:], xsq[:],
                                     start=True, stop=True)
                    sd = pA.tile([1, LC], F32, tag="sd")
                    nc.scalar.activation(sd[:], var[:], AF.Sqrt,
                                         bias=eps_sb[:, 0:1])
                    rstd = pA.tile([1, LC], BF, tag="rstd")
                    with nc.allow_low_precision(reason="bf16 rstd; tol 2e-2"):
                        nc.vector.reciprocal(rstd[:], sd[:])
                    rstdb = psP.tile([128, LC], F32, tag="gen", bufs=2)
                    nc.tensor.matmul(rstdb[:], onesr_sb[:], rstd[:],
                                     start=True, stop=True)
                    nc.vector.tensor_tensor(xnc[:, cs], xcf[:, cs], rstdb[:],
                                            op=ALU.mult)
                # pass2: gate (Sigmoid)
                for c in range(NCH):
                    cs = slice(c * LC, (c + 1) * LC)
                    gps = psP.tile([128, LC], F32, tag="gen", bufs=2)
                    nc.tensor.matmul(gps[:], gateWT_sb[:], xnc[:, cs],
                                     start=True, stop=True)
                    nc.scalar.activation(gate[:, cs], gps[:], AF.Sigmoid,
                                         bias=gateb_sb[:, 0:1])
                # pass3: xz both local groups packed (Silu; Copy free)
                for c in range(NCH):
                    cs = slice(c * LC, (c + 1) * LC)
                    up = psP.tile([128, LC], F32, tag="gen", bufs=2)
                    nc.tensor.matmul(up[:], winTu_sb[:], xnc[:, cs],
                                     start=True, stop=True)
                    nc.scalar.copy(u_pad[:, 3 + c * LC:3 + (c + 1) * LC], up[:])
                    zp = psP.tile([128, LC], F32, tag="gen", bufs=2)
                    nc.tensor.matmul(zp[:], winTz_sb[:], xnc[:, cs],
                                     start=True, stop=True)
                    nc.scalar.activation(zs[:, cs], zp[:], AF.Silu,
                                         bias=zb_sb[:, 0:1])

            # ---- Phase B ----
            # All four (gl,dr) front-ends first (one table set per pass),
            # then the four j-loops (Exp table once).
            with tc.tile_pool(name="pB", bufs=2) as wp:
                uc_all = [bp.tile([128, L], BF, name=f"uc{i4}")
                          for i4 in range(4)]
                dt_all = [bp.tile([128, L], BF, name=f"dtt{i4}")
                          for i4 in range(4)]
                # FE pass1: conv + silu (Silu table)
                for gl in range(2):
                    rows = slice(gl * 64, gl * 64 + 64)
                    for dr in range(2):
                        i4 = gl * 2 + dr
                        uc = uc_all[i4]
                        for cc in range(NCH // 2):
                            ucp = psP.tile([128, 2 * LC], F32, tag="gen", bufs=2)
                            for half in range(2):
                                c = cc * 2 + half
                                for k in range(DC):
                                    off = (c * LC + k) if dr == 0 else (3 + c * LC + k)
                                    nc.tensor.matmul(
                                        ucp[:, half * LC:(half + 1) * LC],
                                        conv4T_sb[rows,
                                                  (i4 * 4 + k) * 128:
                                                  (i4 * 4 + k + 1) * 128],
                                        u_pad[rows, off:off + LC],
                                        start=(k == 0), stop=(k == DC - 1))
                            nc.scalar.activation(
                                uc[:, cc * 2 * LC:(cc + 1) * 2 * LC], ucp[:],
                                AF.Silu, bias=convb_sb[:, i4:i4 + 1])
                # FE pass2: dt matmul + sigmoid (Sigmoid table)
                for i4 in range(4):
                    uc, dt_t = uc_all[i4], dt_all[i4]
                    for cc in range(NCH // 2):
                        c2s = slice(cc * 2 * LC, (cc + 1) * 2 * LC)
                        dtp = psP.tile([128, 2 * LC], F32, tag="gen", bufs=2)
                        for half in range(2):
                            c = cc * 2 + half
                            nc.tensor.matmul(
                                dtp[:, half * LC:(half + 1) * LC],
                                dtWT_sb[:, i4 * 128:(i4 + 1) * 128],
                                uc[0:DI, c * LC:(c + 1) * LC],
                                start=True, stop=True)
                        nc.scalar.activation(dt_t[:, c2s], dtp[:],
                                             AF.Sigmoid,
                                             bias=dtb_sb[:, i4:i4 + 1],
                                             scale=-1.0)
                # FE pass3: dt_t = ln(sigmoid(..)) = -softplus (Ln table)
                for i4 in range(4):
                    nc.scalar.activation(dt_all[i4][:], dt_all[i4][:], AF.Ln)

                # j-loops (Exp table; Copy free)
                for gl in range(2):
                    rows = slice(gl * 64, gl * 64 + 64)
                    for dr in range(2):
                        i4 = gl * 2 + dr
                        uc, dt_t = uc_all[i4], dt_all[i4]
                        dtuc = wp.tile([128, L], BF, tag="dtuc", bufs=2)
                        nc.vector.tensor_tensor(dtuc[:], dt_t[:], uc[:],
                                                op=ALU.mult)
                        horder = (0, 1) if dr == 0 else (1, 0)
                        h_prev = [None] * NJ
                        for hf in horder:
                            hs = slice(hf * LH, (hf + 1) * LH)
                            first = (hf == horder[0])
                            ys = [psP.tile([128, LC], F32, tag=f"ys{q}",
                                           bufs=1, name=f"ys{q}")
                                  for q in range(NCC)]
                            prev_prod = None

                            def bcast(side, j):
                                # B/C broadcast for s-pair j, direct from uc
                                bb = wp.tile([128, LH], BF,
                                             tag=("bbB" if side == 0 else "bbC"))
                                wsl = ((i4 * NJ + j) * 2 + side) * 128
                                for p2 in range(2):
                                    bps = psP.tile([128, 2 * LC], F32,
                                                   tag="gen", bufs=2)
                                    for half in range(2):
                                        q = p2 * 2 + half
                                        nc.tensor.matmul(
                                            bps[:, half * LC:(half + 1) * LC],
                                            xprojJ_sb[:, wsl:wsl + 128],
                                            uc[0:DI, hf * LH + q * LC:
                                               hf * LH + (q + 1) * LC],
                                            start=True, stop=True)
                                    nc.scalar.copy(
                                        bb[:, p2 * 2 * LC:(p2 + 1) * 2 * LC],
                                        bps[:])
                                return bb

                            for j in range(NJ):
                                dA = wp.tile([128, LH], BF, tag="dA")
                                nc.scalar.activation(
                                    dA[:], dt_t[:, hs], AF.Exp,
                                    scale=acol_sb[:, i4 * NJ + j:i4 * NJ + j + 1])
                                bbB = bcast(0, j)
                                dBu = wp.tile([128, LH], BF, tag="dBu")
                                nc.vector.tensor_tensor(dBu[:], dtuc[:, hs],
                                                        bbB[:], op=ALU.mult)
                                h = hp.tile([128, LH], BF, tag="h")
                                hc = hp.tile([128, 1], BF, tag=f"hc{j}",
                                             name=f"hc{j}")
                                init = 0.0 if first else h_prev[j][:, 0:1]
                                if dr == 0:
                                    nc.vector.tensor_tensor_scan(
                                        h[:], dA[:], dBu[:], init,
                                        op0=ALU.mult, op1=ALU.add)
                                    nc.scalar.copy(hc[:], h[:, LH - 1:LH])
                                else:
                                    nc.vector.tensor_tensor_scan(
                                        h[:, ::-1], dA[:, ::-1], dBu[:, ::-1],
                                        init, op0=ALU.mult, op1=ALU.add)
                                    nc.scalar.copy(hc[:], h[:, 0:1])
                                h_prev[j] = hc
                                bbC = bcast(1, j)
                                prod = wp.tile([128, LH], BF, tag="prod")
                                nc.vector.tensor_tensor(prod[:], h[:], bbC[:],
                                                        op=ALU.mult)
                                if prev_prod is not None:
                                    for q in range(NCC):
                                        nc.tensor.matmul(
                                            ys[q][rows, :], mredM_sb[:, 0:DI],
                                            prev_prod[:, q * LC:(q + 1) * LC],
                                            start=(j - 1 == 0), stop=False)
                                prev_prod = prod
                            for q in range(NCC):
                                nc.tensor.matmul(
                                    ys[q][rows, :], mredM_sb[:, 0:DI],
                                    prev_prod[:, q * LC:(q + 1) * LC],
                                    start=(NJ == 1), stop=True)
                            # epilogue for this half
                            for q in range(NCC):
                                c = hf * NCC + q
                                cs = slice(c * LC, (c + 1) * LC)
                                y1 = wp.tile([128, LC], BF, tag="y1")
                                nc.vector.scalar_tensor_tensor(
                                    y1[rows, :], uc[rows, cs],
                                    dsk_sb[rows, i4:i4 + 1],
                                    ys[q][rows, :], op0=ALU.mult,
                                    op1=ALU.subtract)
                                if dr == 0:
                                    nc.gpsimd.tensor_tensor(yfb[rows, cs],
                                                            y1[rows, :],
                                                            zs[rows, cs],
                                                            op=ALU.mult)
                                else:
                                    y2 = wp.tile([128, LC], BF, tag="y2")
                                    nc.gpsimd.tensor_tensor(y2[rows, :],
                                                            y1[rows, :],
                                                            zs[rows, cs],
                                                            op=ALU.mult)
                                    nc.gpsimd.tensor_tensor(yfb[rows, cs],
                                                            yfb[rows, cs],
                                                            y2[rows, :],
                                                            op=ALU.add)

            # ---- Phase C: Wout, exchange, blend, proj ----
            with tc.tile_pool(name="pC", bufs=2) as wpc:
                for c in range(NCH):
                    cs = slice(c * LC, (c + 1) * LC)
                    ymp = psP.tile([2 * DM, LC], F32, tag="gen", bufs=2)
                    nc.tensor.matmul(ymp[:], woutT_sb[:], yfb[:, cs],
                                     start=True, stop=True)
                    ym_sb = wpc.tile([2 * DM, LC], BF, tag="ymsb")
                    nc.scalar.copy(ym_sb[:], ymp[:])
                    nc.sync.dma_start(ym_loc[:, cs], ym_sb[:])
                nc.gpsimd.collective_compute(
                    "AllGather", ALU.bypass,
                    replica_groups=[[0, 1], [2, 3], [4, 5], [6, 7]],
                    ins=[ym_loc[:]], outs=[ym_all[:]])
                for c in range(NCH):
                    cs = slice(c * LC, (c + 1) * LC)
                    xm_t = wpc.tile([C, LC], BF, tag="xmt")
                    nc.sync.dma_start(xm_t[:], ym_all[:, cs])
                    ta = wpc.tile([128, LC], BF, tag="ta")
                    nc.gpsimd.tensor_tensor(ta[:], xm_t[:], xs[:, cs],
                                            op=ALU.subtract)
                    tb2 = wpc.tile([128, LC], BF, tag="tb")
                    nc.vector.tensor_tensor(tb2[:], gate[:, cs], ta[:],
                                            op=ALU.mult)
                    tc2 = wpc.tile([128, LC], BF, tag="tc")
                    nc.gpsimd.tensor_tensor(tc2[:], xs[:, cs], tb2[:],
                                            op=ALU.add)
                    op_ = psP.tile([128, LC], F32, tag="gen", bufs=2)
                    nc.tensor.matmul(op_[:], projT_sb[:], tc2[:],
                                     start=True, stop=True)
                    osb = wpc.tile([128, LC], F32, tag="osb")
                    nc.scalar.activation(osb[:], op_[:], AF.Identity,
                                         bias=projb_sb[:, 0:1])
                    nc.sync.dma_start(outp[:, cs], osb[:])
    nc.finalize()
    return nc


def _bf(a):
    import concourse.mybir as _mb
    return np.asarray(a).astype(_mb.dt.np(_mb.dt.bfloat16))


def _prep_inputs(inputs):
    """Build the 8 per-core in_maps from full inputs."""
    ii = {k: np.asarray(v, dtype=np.float32) for k, v in inputs.items()}
    x = ii["x"]

    maps_w = []  # weight dicts per group-set gs=0,1
    for gs in range(2):
        w = {}
        w9 = np.zeros((C, 9 * 128), np.float32)
        for tap in range(9):
            dy, dx = tap // 3, tap % 3
            blk = np.zeros((C, 128), np.float32)
            np.fill_diagonal(blk, ii["pos_conv_w"][:, 0, dy, dx])
            if tap == 4:
                blk[np.arange(C), np.arange(C)] += 1.0
            w9[:, tap * 128:(tap + 1) * 128] = blk
        w["w9"] = _bf(w9)
        w["pe_b"] = _bf(np.ascontiguousarray(ii["pos_embed"][0].T)
                        + ii["pos_conv_b"][:, None])
        w["mred1"] = _bf(np.full((128, 1), 1.0 / 128, np.float32))
        w["onesr"] = _bf(np.ones((1, 128), np.float32))
        lng = ii["ln_g"]
        lnb = ii["ln_b"]
        # LN affine folded into consumers: xnc on-device is (x-mu)/sd
        w["gateWT"] = _bf(ii["gate_W"].T * lng[:, None])
        w["gateb"] = np.ascontiguousarray(
            (ii["gate_b"] + ii["gate_W"] @ lnb)[:, None])
        w["projT"] = _bf(ii["proj_W"].T)
        w["projb"] = np.ascontiguousarray(ii["proj_b"][:, None])
        w["mredM"] = _bf(np.tile(np.eye(DI, dtype=np.float32), (2, 1)))
        winTu = np.zeros((C, 128), np.float32)
        winTz = np.zeros((C, 128), np.float32)
        zb = np.zeros((128, 1), np.float32)
        ub = np.zeros((128,), np.float32)
        conv4T = np.zeros((2, 2, DC, DI, 128), np.float32)
        convb = np.zeros((2, 2, 128, 1), np.float32)
        dtWT = np.zeros((2, 2, DI, 128), np.float32)
        dtb = np.zeros((2, 2, 128, 1), np.float32)
        xprojJ = np.zeros((2, 2, NJ, 2, DI, 128), np.float32)
        A_col = np.zeros((2, 2, 128, NJ), np.float32)
        dsk = np.zeros((2, 2, 128, 1), np.float32)
        woutT = np.zeros((128, 2 * DM), np.float32)
        for gl in range(2):
            gg = gs * 2 + gl
            gsl = slice(gg * DM, (gg + 1) * DM)
            Wu = ii["m_Win"][gg, 0:DI, :]        # (DI, DM)
            Wz = ii["m_Win"][gg, DI:2 * DI, :]
            winTu[gsl, gl * DI:(gl + 1) * DI] = (Wu * lng[None, gsl]).T
            winTz[gsl, gl * DI:(gl + 1) * DI] = (Wz * lng[None, gsl]).T
            ub[gl * DI:(gl + 1) * DI] = Wu @ lnb[gsl]
            zb[gl * DI:(gl + 1) * DI, 0] = Wz @ lnb[gsl]
            woutT[gl * 64:(gl + 1) * 64, gl * DM:(gl + 1) * DM] = ii["m_Wout"][gg].T
            for dr in range(2):
                for k in range(DC):
                    wk = ii["conv_w"][gg, dr, :, k if dr == 0 else DC - 1 - k]
                    blk = np.zeros((DI, 128), np.float32)
                    blk[np.arange(DI), np.arange(DI)] = wk
                    blk[np.arange(DI), 64 + np.arange(DI)] = wk
                    conv4T[gl, dr, k] = blk
                convb[gl, dr, :, 0] = np.tile(
                    ii["conv_b"][gg, dr]
                    + ii["conv_w"][gg, dr].sum(-1) * ub[gl * DI:(gl + 1) * DI], 2)
                M2 = ii["dt_W"][gg, dr] @ ii["xproj_W"][gg, dr][0:DTR, :]
                dtWT[gl, dr] = np.concatenate([M2.T, M2.T], axis=1)
                dtb[gl, dr, :, 0] = -np.tile(ii["dt_b"][gg, dr], 2)
                Wb = ii["xproj_W"][gg, dr][DTR:DTR + DS, :]        # (DS, DI)
                Wc = ii["xproj_W"][gg, dr][DTR + DS:DTR + 2 * DS, :]
                p = np.arange(128)
                for j in range(NJ):
                    xprojJ[gl, dr, j, 0] = Wb[2 * j + p[None, :] // 64,
                                              np.arange(DI)[:, None]]
                    xprojJ[gl, dr, j, 1] = Wc[2 * j + p[None, :] // 64,
                                              np.arange(DI)[:, None]]
                A = np.exp(ii["A_log"][gg, dr])  # (DI, DS); dt_t is -dt
                for j in range(NJ):
                    A_col[gl, dr, :, j] = A[p % 64, 2 * j + p // 64]
                dsk[gl, dr, :, 0] = np.tile(ii["Dskip"][gg, dr], 2)
        w.update(winTu=_bf(winTu), winTz=_bf(winTz), zb=zb,
                 ub_neg=_bf(np.tile(-ub[:, None], (1, 3))),
                 conv4T=_bf(conv4T), convb=convb, dtWT=_bf(dtWT), dtb=dtb,
                 xprojJ=_bf(xprojJ), A_col=A_col, dsk=dsk, woutT=_bf(woutT))
        maps_w.append(w)

    in_maps = []
    for k in range(NCORE):
        b, gs = k // 2, k % 2
        m = dict(maps_w[gs])
        xp = np.zeros((C, 66, 66), np.float32)
        xp[:, 1:65, 1:65] = x[b]
        m["xpad"] = _bf(np.ascontiguousarray(xp.reshape(C, 66 * 66)))
        in_maps.append(m)
    return in_maps


_CACHE = {}


def kernel(**inputs):
    from concourse.bass_utils import run_bass_kernel_spmd
    if "nc" not in _CACHE:
        _CACHE["nc"] = _build_nc()
    nc = _CACHE["nc"]
    in_maps = _prep_inputs(inputs)
    res = run_bass_kernel_spmd(nc, in_maps, list(range(NCORE))).results
    out = np.stack([np.asarray(res[2 * b]["outp"]).reshape(OUT, H, W)
                    for b in range(B)])
    return out.astype(np.float32)


# revision 34
# speedup vs baseline: 1.3675x; 1.0004x over previous
"""Trainium2 Bass kernel for CDMamba ModifiedSRCMLayer (self-contained).

Sharding: 8 cores; core k handles batch k//2 and mamba group-pair k%2
(groups {0,1} or {2,3}). Group outputs are exchanged with a paired
AllGather; the post-stage (gate blend + output projection) is computed
redundantly on both cores of a pair and the host reads even cores.

v2: all-bf16 datapath. Phase A/C in bf16 packed matmuls, grouped into
activation-table passes (Sqrt | Sigmoid | Silu). Phase B front-end uses
AF.Silu / AF.Softplus directly; the j-loop broadcasts B/C with PE
matmuls, copies PSUM->SBUF bf16 on the ACT engine, and runs dBu/prod as
[128,2048] pure-bf16 DVE tensor_tensor ops (16-bit 2x rate) plus the
DVE tensor_tensor_scan. The s-reduction accumulates in PSUM via matmul.
"""
import sys
import numpy as np

for _p in ("/opt/trn_rl_repo",):
    if _p not in sys.path:
        sys.path.append(_p)

import concourse.bass as bass
import concourse.mybir as mybir
from concourse.bacc import Bacc
from concourse.tile import TileContext

# Model dims (hardcoded per the problem spec)
B, C, H, W = 4, 128, 64, 64
L = H * W                      # 4096
G, DM = 4, 32
DI, DS, DC = 64, 16, 4
DTR = 2
OUT = 128
EPS = 1e-5

NCORE = 8
LC = 512
NCH = L // LC                  # 8
LH = L // 2                    # 2048
NCC = LH // LC                 # 4
NJ = DS // 2                   # 8 j-tiles (2 s-values per tile)
F32 = mybir.dt.float32
BF = mybir.dt.bfloat16
AF = mybir.ActivationFunctionType
ALU = mybir.AluOpType


def _build_nc():
    nc = Bacc(num_devices=NCORE)

    def inp(name, shape, dt=BF):
        return nc.dram_tensor(name, list(shape), dt, kind="ExternalInput")

    xpad = inp("xpad", (C, 66 * 66))
    pe_b = inp("pe_b", (C, L))
    w9 = inp("w9", (C, 9 * 128))
    mred1 = inp("mred1", (128, 1))
    onesr = inp("onesr", (1, 128))
    gateWT = inp("gateWT", (128, 128))
    gateb = inp("gateb", (128, 1), F32)
    winTu = inp("winTu", (C, 128))       # u for both local groups
    winTz = inp("winTz", (C, 128))
    zb = inp("zb", (128, 1), F32)        # ln-affine fold: Win_z @ ln_b
    ub_neg = inp("ub_neg", (128, 3))     # -Win_u @ ln_b (conv halo)
    conv4T = inp("conv4T", (2, 2, DC, DI, 128))
    convb = inp("convb", (2, 2, 128, 1), F32)
    dtWT = inp("dtWT", (2, 2, DI, 128))
    dtb = inp("dtb", (2, 2, 128, 1), F32)
    xprojJ = inp("xprojJ", (2, 2, NJ, 2, DI, 128))  # replicated B/C weights
    A_col = inp("A_col", (2, 2, 128, NJ), F32)
    dsk = inp("dsk", (2, 2, 128, 1), F32)
    mredM = inp("mredM", (128, DI))
    woutT = inp("woutT", (128, 2 * DM))
    projT = inp("projT", (128, 128))
    projb = inp("projb", (128, 1), F32)

    ym_loc = nc.dram_tensor("ym_loc", [2 * DM, L], BF)
    ym_all = nc.dram_tensor("ym_all", [C, L], BF)
    outp = nc.dram_tensor("outp", [OUT, L], F32, kind="ExternalOutput")

    with TileContext(nc) as tc:
        with (
            tc.tile_pool(name="const", bufs=1) as cp,
            tc.tile_pool(name="big", bufs=1) as bp,
            tc.tile_pool(name="hpool", bufs=2) as hp,
            tc.tile_pool(name="psP", bufs=1, space="PSUM") as psP,
        ):
            # ---- constants to SBUF ----
            def c_load(ap_dram, shape, nm, dt=BF):
                t = cp.tile(list(shape), dt, name=nm, tag=nm)
                nc.sync.dma_start(t[:], ap_dram)
                return t

            w9_sb = c_load(w9[:], (C, 9 * 128), "w9sb")
            mred1_sb = c_load(mred1[:], (128, 1), "mred1sb")
            onesr_sb = c_load(onesr[:], (1, 128), "onesrsb")
            gateWT_sb = c_load(gateWT[:], (128, 128), "gateWTsb")
            gateb_sb = c_load(gateb[:], (128, 1), "gatebsb", F32)
            winTu_sb = c_load(winTu[:], (C, 128), "winTusb")
            winTz_sb = c_load(winTz[:], (C, 128), "winTzsb")
            zb_sb = c_load(zb[:], (128, 1), "zbsb", F32)
            ubn_sb = c_load(ub_neg[:], (128, 3), "ubnsb")
            mredM_sb = c_load(mredM[:], (128, DI), "mredMsb")
            woutT_sb = c_load(woutT[:], (128, 2 * DM), "woutTsb")
            projT_sb = c_load(projT[:], (128, 128), "projTsb")
            projb_sb = c_load(projb[:], (128, 1), "projbsb", F32)

            conv4T_sb = cp.tile([128, 16 * 128], BF)
            dtWT_sb = cp.tile([DI, 4 * 128], BF)
            xprojJ_sb = cp.tile([DI, 4 * NJ * 2 * 128], BF)
            acol_sb = cp.tile([128, 4 * NJ], F32)
            convb_sb = cp.tile([128, 4], F32)
            dtb_sb = cp.tile([128, 4], F32)
            dsk_sb = cp.tile([128, 4], F32)
            eps_sb = cp.tile([1, 1], F32)
            nc.vector.memset(eps_sb[:], EPS)
            for gl in range(2):
                for dr in range(2):
                    i4 = gl * 2 + dr
                    for k in range(DC):
                        for hh in range(2):
                            nc.sync.dma_start(
                                conv4T_sb[hh * 64:(hh + 1) * 64,
                                          (i4 * 4 + k) * 128:(i4 * 4 + k + 1) * 128],
                                conv4T[gl, dr, k])
                    nc.sync.dma_start(dtWT_sb[:, i4 * 128:(i4 + 1) * 128],
                                      dtWT[gl, dr])
                    for j in range(NJ):
                        for sd_ in range(2):
                            o = ((i4 * NJ + j) * 2 + sd_) * 128
                            nc.sync.dma_start(xprojJ_sb[:, o:o + 128],
                                              xprojJ[gl, dr, j, sd_])
                    nc.sync.dma_start(acol_sb[:, i4 * NJ:(i4 + 1) * NJ],
                                      A_col[gl, dr])
                    nc.sync.dma_start(convb_sb[:, i4:i4 + 1], convb[gl, dr])
                    nc.sync.dma_start(dtb_sb[:, i4:i4 + 1], dtb[gl, dr])
                    nc.sync.dma_start(dsk_sb[:, i4:i4 + 1], dsk[gl, dr])

            # ---- persistent tiles ----
            xs = bp.tile([C, L], BF)
            gate = bp.tile([C, L], BF)
            u_pad = bp.tile([C, L + 6], BF)
            zs = bp.tile([C, L], BF)
            yfb = bp.tile([C, L], BF)

            # halo = -Win_u@ln_b so the folded-LN conv matches zero-padded ref
            nc.vector.tensor_copy(u_pad[:, 0:3], ubn_sb[:])
            nc.vector.tensor_copy(u_pad[:, L + 3:L + 6], ubn_sb[:])

            # ---- Phase A ----
            with tc.tile_pool(name="pA", bufs=2) as pA:
                xpad_sb = pA.tile([C, 66 * 66], BF, bufs=1)
                nc.sync.dma_start(xpad_sb[:], xpad[:])
                xpad3 = xpad_sb[:].rearrange("p (r q) -> p r q", q=66)
                xnc = pA.tile([C, L], BF, bufs=1)   # centered/normed (LN affine folded)
                xcf = pA.tile([C, L], BF, bufs=1)   # centered
                # pass0: pos-enc conv, dense PE burst (no tables)
                for c in range(NCH):
                    cs = slice(c * LC, (c + 1) * LC)
                    pa = psP.tile([128, 8, 64], F32, tag="gen", bufs=2)
                    for tap in range(9):
                        dy, dx = tap // 3, tap % 3
                        nc.tensor.matmul(
                            pa[:],
                            w9_sb[:, tap * 128:(tap + 1) * 128],
                            xpad3[:, c * 8 + dy:c * 8 + dy + 8, dx:dx + 64],
                            start=(tap == 0), stop=(tap == 8))
                    paf = pa[:].rearrange("p a b -> p (a b)")
                    pe_t = pA.tile([128, LC], BF, tag="pe")
                    nc.sync.dma_start(pe_t[:], pe_b[:, cs])
                    nc.vector.tensor_tensor(xs[:, cs], paf, pe_t[:], op=ALU.add)
                # pass1: LN in 1024-wide stages (tables: Sqrt; Square free)
                for c2 in range(NCH // 2):
                    c2s = slice(c2 * 2 * LC, (c2 + 1) * 2 * LC)
                    mu = psP.tile([1, 2 * LC], F32, tag="mu", bufs=2)
                    for half in range(2):
                        nc.tensor.matmul(
                            mu[:, half * LC:(half + 1) * LC], mred1_sb[:],
                            xs[:, (c2 * 2 + half) * LC:(c2 * 2 + half + 1) * LC],
                            start=True, stop=True)
                    mu_sb = pA.tile([1, 2 * LC], BF, tag="musb")
                    nc.scalar.copy(mu_sb[:], mu[:])
                    mub = psP.tile([128, 2 * LC], F32, tag="gen", bufs=2)
                    for half in range(2):
                        nc.tensor.matmul(
                            mub[:, half * LC:(half + 1) * LC], onesr_sb[:],
                            mu_sb[:, half * LC:(half + 1) * LC],
                            start=True, stop=True)
                    nc.vector.tensor_tensor(xcf[:, c2s], xs[:, c2s], mub[:],
                                            op=ALU.subtract)
                    xsq = pA.tile([128, 2 * LC], BF, tag="xsq")
                    nc.vector.tensor_tensor(xsq[:], xcf[:, c2s], xcf[:, c2s],
                                            op=ALU.mult)
                    var = psP.tile([1, 2 * LC], F32, tag="mu", bufs=2)
                    for half in range(2):
                        nc.tensor.matmul(
                            var[:, half * LC:(half + 1) * LC], mred1_sb[:],
                            xsq[:, half * LC:(half + 1) * LC],
                            start=True, stop=True)
                    sd = pA.tile([1, 2 * LC], F32, tag="sd")
                    nc.scalar.activation(sd[:], var[:], AF.Sqrt,
                                         bias=eps_sb[:, 0:1])
                    rstd = pA.tile([1, 2 * LC], BF, tag="rstd")
                    with nc.allow_low_precision(reason="bf16 rstd; tol 2e-2"):
                        nc.vector.reciprocal(rstd[:], sd[:])
                    rstdb = psP.tile([128, 2 * LC], F32, tag="gen", bufs=2)
                    for half in range(2):
                        nc.tensor.matmul(
                            rstdb[:, half * LC:(half + 1) * LC], onesr_sb[:],
                            rstd[:, half * LC:(half + 1) * LC],
                            start=True, stop=True)
                    nc.vector.tensor_tensor(xnc[:, c2s], xcf[:, c2s], rstdb[:],
                                            op=ALU.mult)
                # pass2: gate (Sigmoid)
                for c in range(NCH):
                    cs = slice(c * LC, (c + 1) * LC)
                    gps = psP.tile([128, LC], F32, tag="gen", bufs=2)
                    nc.tensor.matmul(gps[:], gateWT_sb[:], xnc[:, cs],
                                     start=True, stop=True)
                    nc.scalar.activation(gate[:, cs], gps[:], AF.Sigmoid,
                                         bias=gateb_sb[:, 0:1])
                # pass3: xz both local groups packed (Silu; Copy free)
                for c in range(NCH):
                    cs = slice(c * LC, (c + 1) * LC)
                    up = psP.tile([128, LC], F32, tag="gen", bufs=2)
                    nc.tensor.matmul(up[:], winTu_sb[:], xnc[:, cs],
                                     start=True, stop=True)
                    nc.scalar.copy(u_pad[:, 3 + c * LC:3 + (c + 1) * LC], up[:])
                    zp = psP.tile([128, LC], F32, tag="gen", bufs=2)
                    nc.tensor.matmul(zp[:], winTz_sb[:], xnc[:, cs],
                                     start=True, stop=True)
                    nc.scalar.activation(zs[:, cs], zp[:], AF.Silu,
                                         bias=zb_sb[:, 0:1])

            # ---- Phase B ----
            # All four (gl,dr) front-ends first (one table set per pass),
            # then the four j-loops (Exp table once).
            with tc.tile_pool(name="pB", bufs=2) as wp:
                uc_all = [bp.tile([128, L], BF, name=f"uc{i4}")
                          for i4 in range(4)]
                dt_all = [bp.tile([128, L], BF, name=f"dtt{i4}")
                          for i4 in range(4)]
                # FE pass1: conv + silu (Silu table)
                for gl in range(2):
                    rows = slice(gl * 64, gl * 64 + 64)
                    for dr in range(2):
                        i4 = gl * 2 + dr
                        uc = uc_all[i4]
                        for cc in range(NCH // 2):
                            ucp = psP.tile([128, 2 * LC], F32, tag="gen", bufs=2)
                            for half in range(2):
                                c = cc * 2 + half
                                for k in range(DC):
                                    off = (c * LC + k) if dr == 0 else (3 + c * LC + k)
                                    nc.tensor.matmul(
                                        ucp[:, half * LC:(half + 1) * LC],
                                        conv4T_sb[rows,
                                                  (i4 * 4 + k) * 128:
                                                  (i4 * 4 + k + 1) * 128],
                                        u_pad[rows, off:off + LC],
                                        start=(k == 0), stop=(k == DC - 1))
                            nc.scalar.activation(
                                uc[:, cc * 2 * LC:(cc + 1) * 2 * LC], ucp[:],
                                AF.Silu, bias=convb_sb[:, i4:i4 + 1])
                # FE pass2: dt matmul + sigmoid (Sigmoid table)
                for i4 in range(4):
                    uc, dt_t = uc_all[i4], dt_all[i4]
                    for cc in range(NCH // 2):
                        c2s = slice(cc * 2 * LC, (cc + 1) * 2 * LC)
                        dtp = psP.tile([128, 2 * LC], F32, tag="gen", bufs=2)
                        for half in range(2):
                            c = cc * 2 + half
                            nc.tensor.matmul(
                                dtp[:, half * LC:(half + 1) * LC],
                                dtWT_sb[:, i4 * 128:(i4 + 1) * 128],
                                uc[0:DI, c * LC:(c + 1) * LC],
                                start=True, stop=True)
                        nc.scalar.activation(dt_t[:, c2s], dtp[:],
                                             AF.Sigmoid,
                                             bias=dtb_sb[:, i4:i4 + 1],
                                             scale=-1.0)
                # FE pass3: dt_t = ln(sigmoid(..)) = -softplus (Ln table)
                for i4 in range(4):
                    nc.scalar.activation(dt_all[i4][:], dt_all[i4][:], AF.Ln)

                # j-loops (Exp table; Copy free)
                for gl in range(2):
                    rows = slice(gl * 64, gl * 64 + 64)
                    for dr in range(2):
                        i4 = gl * 2 + dr
                        uc, dt_t = uc_all[i4], dt_all[i4]
                        dtuc = wp.tile([128, L], BF, tag="dtuc", bufs=2)
                        nc.vector.tensor_tensor(dtuc[:], dt_t[:], uc[:],
                                                op=ALU.mult)
                        horder = (0, 1) if dr == 0 else (1, 0)
                        h_prev = [None] * NJ
                        for hf in horder:
                            hs = slice(hf * LH, (hf + 1) * LH)
                            first = (hf == horder[0])
                            ys = [psP.tile([128, LC], F32, tag=f"ys{q}",
                                           bufs=1, name=f"ys{q}")
                                  for q in range(NCC)]
                            prev_prod = None

                            def bcast(side, j):
                                # B/C broadcast for s-pair j, direct from uc
                                bb = wp.tile([128, LH], BF, bufs=3,
                                             tag=("bbB" if side == 0 else "bbC"))
                                wsl = ((i4 * NJ + j) * 2 + side) * 128
                                for p2 in range(2):
                                    bps = psP.tile([128, 2 * LC], F32,
                                                   tag="gen", bufs=2)
                                    for half in range(2):
                                        q = p2 * 2 + half
                                        nc.tensor.matmul(
                                            bps[:, half * LC:(half + 1) * LC],
                                            xprojJ_sb[:, wsl:wsl + 128],
                                            uc[0:DI, hf * LH + q * LC:
                                               hf * LH + (q + 1) * LC],
                                            start=True, stop=True)
                                    nc.scalar.copy(
                                        bb[:, p2 * 2 * LC:(p2 + 1) * 2 * LC],
                                        bps[:])
                                return bb

                            for j in range(NJ):
                                dA = wp.tile([128, LH], BF, tag="dA", bufs=2)
                                nc.scalar.activation(
                                    dA[:], dt_t[:, hs], AF.Exp,
                                    scale=acol_sb[:, i4 * NJ + j:i4 * NJ + j + 1])
                                bbB = bcast(0, j)
                                dBu = wp.tile([128, LH], BF, tag="dBu")
                                nc.vector.tensor_tensor(dBu[:], dtuc[:, hs],
                                                        bbB[:], op=ALU.mult)
                                h = hp.tile([128, LH], BF, tag="h")
                                hc = hp.tile([128, 1], BF, tag=f"hc{j}",
                                             name=f"hc{j}")
                                init = 0.0 if first else h_prev[j][:, 0:1]
                                if dr == 0:
                                    nc.vector.tensor_tensor_scan(
                                        h[:], dA[:], dBu[:], init,
                                        op0=ALU.mult, op1=ALU.add)
                                    nc.gpsimd.tensor_copy(hc[:], h[:, LH - 1:LH])
                                else:
                                    nc.vector.tensor_tensor_scan(
                                        h[:, ::-1], dA[:, ::-1], dBu[:, ::-1],
                                        init, op0=ALU.mult, op1=ALU.add)
                                    nc.gpsimd.tensor_copy(hc[:], h[:, 0:1])
                                h_prev[j] = hc
                                bbC = bcast(1, j)
                                prod = wp.tile([128, LH], BF, tag="prod")
                                nc.vector.tensor_tensor(prod[:], h[:], bbC[:],
                                                        op=ALU.mult)
                                if prev_prod is not None:
                                    for q in range(NCC):
                                        nc.tensor.matmul(
                                            ys[q][rows, :], mredM_sb[:, 0:DI],
                                            prev_prod[:, q * LC:(q + 1) * LC],
                                            start=(j - 1 == 0), stop=False)
                                prev_prod = prod
                            for q in range(NCC):
                                nc.tensor.matmul(
                                    ys[q][rows, :], mredM_sb[:, 0:DI],
                                    prev_prod[:, q * LC:(q + 1) * LC],
                                    start=(NJ == 1), stop=True)
                            # epilogue for this half
                            for q in range(NCC):
                                c = hf * NCC + q
                                cs = slice(c * LC, (c + 1) * LC)
                                y1 = wp.tile([128, LC], BF, tag="y1")
                                nc.vector.scalar_tensor_tensor(
                                    y1[rows, :], uc[rows, cs],
                                    dsk_sb[rows, i4:i4 + 1],
                                    ys[q][rows, :], op0=ALU.mult,
                                    op1=ALU.subtract)
                                if dr == 0:
                                    nc.gpsimd.tensor_tensor(yfb[rows, cs],
                                                            y1[rows, :],
                                                            zs[rows, cs],
                                                            op=ALU.mult)
                                else:
                                    y2 = wp.tile([128, LC], BF, tag="y2")
                                    nc.gpsimd.tensor_tensor(y2[rows, :],
                                                            y1[rows, :],
                                                            zs[rows, cs],
                                                            op=ALU.mult)
                                    nc.gpsimd.tensor_tensor(yfb[rows, cs],
                                                            yfb[rows, cs],
                                                            y2[rows, :],
                                                            op=ALU.add)

            # ---- Phase C: Wout, exchange, blend, proj ----
            with tc.tile_pool(name="pC", bufs=2) as wpc:
                for c in range(NCH):
                    cs = slice(c * LC, (c + 1) * LC)
                    ymp = psP.tile([2 * DM, LC], F32, tag="gen", bufs=2)
                    nc.tensor.matmul(ymp[:], woutT_sb[:], yfb[:, cs],
                                     start=True, stop=True)
                    ym_sb = wpc.tile([2 * DM, LC], BF, tag="ymsb")
                    nc.scalar.copy(ym_sb[:], ymp[:])
                    nc.sync.dma_start(ym_loc[:, cs], ym_sb[:])
                nc.gpsimd.collective_compute(
                    "AllGather", ALU.bypass,
                    replica_groups=[[0, 1], [2, 3], [4, 5], [6, 7]],
                    ins=[ym_loc[:]], outs=[ym_all[:]])
                for c in range(NCH):
                    cs = slice(c * LC, (c + 1) * LC)
                    xm_t = wpc.tile([C, LC], BF, tag="xmt")
                    nc.sync.dma_start(xm_t[:], ym_all[:, cs])
                    ta = wpc.tile([128, LC], BF, tag="ta")
                    nc.gpsimd.tensor_tensor(ta[:], xm_t[:], xs[:, cs],
                                            op=ALU.subtract)
                    tb2 = wpc.tile([128, LC], BF, tag="tb")
                    nc.vector.tensor_tensor(tb2[:], gate[:, cs], ta[:],
                                            op=ALU.mult)
                    tc2 = wpc.tile([128, LC], BF, tag="tc")
                    nc.gpsimd.tensor_tensor(tc2[:], xs[:, cs], tb2[:],
                                            op=ALU.add)
                    op_ = psP.tile([128, LC], F32, tag="gen", bufs=2)
                    nc.tensor.matmul(op_[:], projT_sb[:], tc2[:],
                                     start=True, stop=True)
                    osb = wpc.tile([128, LC], F32, tag="osb")
                    nc.scalar.activation(osb[:], op_[:], AF.Identity,
                                         bias=projb_sb[:, 0:1])
                    nc.sync.dma_start(outp[:, cs], osb[:])
    nc.finalize()
    return nc


def _bf(a):
    import concourse.mybir as _mb
    return np.asarray(a).astype(_mb.dt.np(_mb.dt.bfloat16))


def _prep_inputs(inputs):
    """Build the 8 per-core in_maps from full inputs."""
    ii = {k: np.asarray(v, dtype=np.float32) for k, v in inputs.items()}
    x = ii["x"]

    maps_w = []  # weight dicts per group-set gs=0,1
    for gs in range(2):
        w = {}
        w9 = np.zeros((C, 9 * 128), np.float32)
        for tap in range(9):
            dy, dx = tap // 3, tap % 3
            blk = np.zeros((C, 128), np.float32)
            np.fill_diagonal(blk, ii["pos_conv_w"][:, 0, dy, dx])
            if tap == 4:
                blk[np.arange(C), np.arange(C)] += 1.0
            w9[:, tap * 128:(tap + 1) * 128] = blk
        w["w9"] = _bf(w9)
        w["pe_b"] = _bf(np.ascontiguousarray(ii["pos_embed"][0].T)
                        + ii["pos_conv_b"][:, None])
        w["mred1"] = _bf(np.full((128, 1), 1.0 / 128, np.float32))
        w["onesr"] = _bf(np.ones((1, 128), np.float32))
        lng = ii["ln_g"]
        lnb = ii["ln_b"]
        # LN affine folded into consumers: xnc on-device is (x-mu)/sd
        w["gateWT"] = _bf(ii["gate_W"].T * lng[:, None])
        w["gateb"] = np.ascontiguousarray(
            (ii["gate_b"] + ii["gate_W"] @ lnb)[:, None])
        w["projT"] = _bf(ii["proj_W"].T)
        w["projb"] = np.ascontiguousarray(ii["proj_b"][:, None])
        w["mredM"] = _bf(np.tile(np.eye(DI, dtype=np.float32), (2, 1)))
        winTu = np.zeros((C, 128), np.float32)
        winTz = np.zeros((C, 128), np.float32)
        zb = np.zeros((128, 1), np.float32)
        ub = np.zeros((128,), np.float32)
        conv4T = np.zeros((2, 2, DC, DI, 128), np.float32)
        convb = np.zeros((2, 2, 128, 1), np.float32)
        dtWT = np.zeros((2, 2, DI, 128), np.float32)
        dtb = np.zeros((2, 2, 128, 1), np.float32)
        xprojJ = np.zeros((2, 2, NJ, 2, DI, 128), np.float32)
        A_col = np.zeros((2, 2, 128, NJ), np.float32)
        dsk = np.zeros((2, 2, 128, 1), np.float32)
        woutT = np.zeros((128, 2 * DM), np.float32)
        for gl in range(2):
            gg = gs * 2 + gl
            gsl = slice(gg * DM, (gg + 1) * DM)
            Wu = ii["m_Win"][gg, 0:DI, :]        # (DI, DM)
            Wz = ii["m_Win"][gg, DI:2 * DI, :]
            winTu[gsl, gl * DI:(gl + 1) * DI] = (Wu * lng[None, gsl]).T
            winTz[gsl, gl * DI:(gl + 1) * DI] = (Wz * lng[None, gsl]).T
            ub[gl * DI:(gl + 1) * DI] = Wu @ lnb[gsl]
            zb[gl * DI:(gl + 1) * DI, 0] = Wz @ lnb[gsl]
            woutT[gl * 64:(gl + 1) * 64, gl * DM:(gl + 1) * DM] = ii["m_Wout"][gg].T
            for dr in range(2):
                for k in range(DC):
                    wk = ii["conv_w"][gg, dr, :, k if dr == 0 else DC - 1 - k]
                    blk = np.zeros((DI, 128), np.float32)
                    blk[np.arange(DI), np.arange(DI)] = wk
                    blk[np.arange(DI), 64 + np.arange(DI)] = wk
                    conv4T[gl, dr, k] = blk
                convb[gl, dr, :, 0] = np.tile(
                    ii["conv_b"][gg, dr]
                    + ii["conv_w"][gg, dr].sum(-1) * ub[gl * DI:(gl + 1) * DI], 2)
                M2 = ii["dt_W"][gg, dr] @ ii["xproj_W"][gg, dr][0:DTR, :]
                dtWT[gl, dr] = np.concatenate([M2.T, M2.T], axis=1)
                dtb[gl, dr, :, 0] = -np.tile(ii["dt_b"][gg, dr], 2)
                Wb = ii["xproj_W"][gg, dr][DTR:DTR + DS, :]        # (DS, DI)
                Wc = ii["xproj_W"][gg, dr][DTR + DS:DTR + 2 * DS, :]
                p = np.arange(128)
                for j in range(NJ):
                    xprojJ[gl, dr, j, 0] = Wb[2 * j + p[None, :] // 64,
                                              np.arange(DI)[:, None]]
                    xprojJ[gl, dr, j, 1] = Wc[2 * j + p[None, :] // 64,
                                              np.arange(DI)[:, None]]
                A = np.exp(ii["A_log"][gg, dr])  # (DI, DS); dt_t is -dt
                for j in range(NJ):
                    A_col[gl, dr, :, j] = A[p % 64, 2 * j + p // 64]
                dsk[gl, dr, :, 0] = np.tile(ii["Dskip"][gg, dr], 2)
        w.update(winTu=_bf(winTu), winTz=_bf(winTz), zb=zb,
                 ub_neg=_bf(np.tile(-ub[:, None], (1, 3))),
                 conv4T=_bf(conv4T), convb=convb, dtWT=_bf(dtWT), dtb=dtb,
                 xprojJ=_bf(xprojJ), A_col=A_col, dsk=dsk, woutT=_bf(woutT))
        maps_w.append(w)

    in_maps = []
    for k in range(NCORE):
        b, gs = k // 2, k % 2
        m = dict(maps_w[gs])
        xp = np.zeros((C, 66, 66), np.float32)
        xp[:, 1:65, 1:65] = x[b]
        m["xpad"] = _bf(np.ascontiguousarray(xp.reshape(C, 66 * 66)))
        in_maps.append(m)
    return in_maps


_CACHE = {}


def kernel(**inputs):
    from concourse.bass_utils import run_bass_kernel_spmd
    if "nc" not in _CACHE:
        _CACHE["nc"] = _build_nc()
    nc = _CACHE["nc"]
    in_maps = _prep_inputs(inputs)
    res = run_bass_kernel_spmd(nc, in_maps, list(range(NCORE))).results
    out = np.stack([np.asarray(res[2 * b]["outp"]).reshape(OUT, H, W)
                    for b in range(B)])
    return out.astype(np.float32)


# revision 37
# speedup vs baseline: 1.3857x; 1.0134x over previous
"""Trainium2 Bass kernel for CDMamba ModifiedSRCMLayer (self-contained).

Sharding: 8 cores; core k handles batch k//2 and mamba group-pair k%2
(groups {0,1} or {2,3}). Group outputs are exchanged with a paired
AllGather; the post-stage (gate blend + output projection) is computed
redundantly on both cores of a pair and the host reads even cores.

v2: all-bf16 datapath. Phase A/C in bf16 packed matmuls, grouped into
activation-table passes (Sqrt | Sigmoid | Silu). Phase B front-end uses
AF.Silu / AF.Softplus directly; the j-loop broadcasts B/C with PE
matmuls, copies PSUM->SBUF bf16 on the ACT engine, and runs dBu/prod as
[128,2048] pure-bf16 DVE tensor_tensor ops (16-bit 2x rate) plus the
DVE tensor_tensor_scan. The s-reduction accumulates in PSUM via matmul.
"""
import sys
import numpy as np

for _p in ("/opt/trn_rl_repo",):
    if _p not in sys.path:
        sys.path.append(_p)

import concourse.bass as bass
import concourse.mybir as mybir
from concourse.bacc import Bacc
from concourse.tile import TileContext

# Model dims (hardcoded per the problem spec)
B, C, H, W = 4, 128, 64, 64
L = H * W                      # 4096
G, DM = 4, 32
DI, DS, DC = 64, 16, 4
DTR = 2
OUT = 128
EPS = 1e-5

NCORE = 8
LC = 512
NCH = L // LC                  # 8
LH = L // 2                    # 2048
NCC = LH // LC                 # 4
NJ = DS // 2                   # 8 j-tiles (2 s-values per tile)
F32 = mybir.dt.float32
BF = mybir.dt.bfloat16
AF = mybir.ActivationFunctionType
ALU = mybir.AluOpType


def _build_nc():
    nc = Bacc(num_devices=NCORE)

    def inp(name, shape, dt=BF):
        return nc.dram_tensor(name, list(shape), dt, kind="ExternalInput")

    xpad = inp("xpad", (C, 66 * 66))
    pe_b = inp("pe_b", (C, L))
    w9 = inp("w9", (C, 9 * 128))
    mred1 = inp("mred1", (128, 1))
    onesr = inp("onesr", (1, 128))
    gateWT = inp("gateWT", (128, 128))
    gateb = inp("gateb", (128, 1), F32)
    winTu = inp("winTu", (C, 128))       # u for both local groups
    winTz = inp("winTz", (C, 128))
    zb = inp("zb", (128, 1), F32)        # ln-affine fold: Win_z @ ln_b
    ub_neg = inp("ub_neg", (128, 3))     # -Win_u @ ln_b (conv halo)
    conv4T = inp("conv4T", (2, 2, DC, DI, 128))
    convb = inp("convb", (2, 2, 128, 1), F32)
    dtWT = inp("dtWT", (2, 2, DI, 128))
    dtb = inp("dtb", (2, 2, 128, 1), F32)
    xprojJ = inp("xprojJ", (2, 2, NJ, 2, DI, 128))  # replicated B/C weights
    A_col = inp("A_col", (2, 2, 128, NJ), F32)
    dsk = inp("dsk", (2, 2, 128, 1), F32)
    mredM = inp("mredM", (128, DI))
    woutT = inp("woutT", (128, 2 * DM))
    projT = inp("projT", (128, 128))
    projb = inp("projb", (128, 1), F32)

    ym_loc = nc.dram_tensor("ym_loc", [2 * DM, L], BF)
    ym_all = nc.dram_tensor("ym_all", [C, L], BF)
    outp = nc.dram_tensor("outp", [OUT, L], F32, kind="ExternalOutput")

    with TileContext(nc) as tc:
        with (
            tc.tile_pool(name="const", bufs=1) as cp,
            tc.tile_pool(name="big", bufs=1) as bp,
            tc.tile_pool(name="hpool", bufs=2) as hp,
            tc.tile_pool(name="psP", bufs=1, space="PSUM") as psP,
        ):
            # ---- constants to SBUF ----
            def c_load(ap_dram, shape, nm, dt=BF):
                t = cp.tile(list(shape), dt, name=nm, tag=nm)
                nc.sync.dma_start(t[:], ap_dram)
                return t

            # w9/xpad first (first conv depends on them), split across queues
            w9_sb = cp.tile([C, 9 * 128], BF, name="w9sb", tag="w9sb")
            for qq in range(4):
                eng = (nc.sync, nc.scalar, nc.vector, nc.gpsimd)[qq]
                eng.dma_start(w9_sb[:, qq * 288:(qq + 1) * 288],
                              w9[:, qq * 288:(qq + 1) * 288])
            mred1_sb = c_load(mred1[:], (128, 1), "mred1sb")
            onesr_sb = c_load(onesr[:], (1, 128), "onesrsb")
            gateWT_sb = c_load(gateWT[:], (128, 128), "gateWTsb")
            gateb_sb = c_load(gateb[:], (128, 1), "gatebsb", F32)
            winTu_sb = c_load(winTu[:], (C, 128), "winTusb")
            winTz_sb = c_load(winTz[:], (C, 128), "winTzsb")
            zb_sb = c_load(zb[:], (128, 1), "zbsb", F32)
            ubn_sb = c_load(ub_neg[:], (128, 3), "ubnsb")
            mredM_sb = c_load(mredM[:], (128, DI), "mredMsb")
            woutT_sb = c_load(woutT[:], (128, 2 * DM), "woutTsb")
            projT_sb = c_load(projT[:], (128, 128), "projTsb")
            projb_sb = c_load(projb[:], (128, 1), "projbsb", F32)

            conv4T_sb = cp.tile([128, 16 * 128], BF)
            dtWT_sb = cp.tile([DI, 4 * 128], BF)
            xprojJ_sb = cp.tile([DI, 4 * NJ * 2 * 128], BF)
            acol_sb = cp.tile([128, 4 * NJ], F32)
            convb_sb = cp.tile([128, 4], F32)
            dtb_sb = cp.tile([128, 4], F32)
            dsk_sb = cp.tile([128, 4], F32)
            eps_sb = cp.tile([1, 1], F32)
            nc.vector.memset(eps_sb[:], EPS)
            for gl in range(2):
                for dr in range(2):
                    i4 = gl * 2 + dr
                    for k in range(DC):
                        for hh in range(2):
                            nc.sync.dma_start(
                                conv4T_sb[hh * 64:(hh + 1) * 64,
                                          (i4 * 4 + k) * 128:(i4 * 4 + k + 1) * 128],
                                conv4T[gl, dr, k])
                    nc.sync.dma_start(dtWT_sb[:, i4 * 128:(i4 + 1) * 128],
                                      dtWT[gl, dr])
                    for j in range(NJ):
                        for sd_ in range(2):
                            o = ((i4 * NJ + j) * 2 + sd_) * 128
                            nc.sync.dma_start(xprojJ_sb[:, o:o + 128],
                                              xprojJ[gl, dr, j, sd_])
                    nc.sync.dma_start(acol_sb[:, i4 * NJ:(i4 + 1) * NJ],
                                      A_col[gl, dr])
                    nc.sync.dma_start(convb_sb[:, i4:i4 + 1], convb[gl, dr])
                    nc.sync.dma_start(dtb_sb[:, i4:i4 + 1], dtb[gl, dr])
                    nc.sync.dma_start(dsk_sb[:, i4:i4 + 1], dsk[gl, dr])

            # ---- persistent tiles ----
            xs = bp.tile([C, L], BF)
            gate = bp.tile([C, L], BF)
            u_pad = bp.tile([C, L + 6], BF)
            zs = bp.tile([C, L], BF)
            yfb = bp.tile([C, L], BF)

            # halo = -Win_u@ln_b so the folded-LN conv matches zero-padded ref
            nc.vector.tensor_copy(u_pad[:, 0:3], ubn_sb[:])
            nc.vector.tensor_copy(u_pad[:, L + 3:L + 6], ubn_sb[:])

            # ---- Phase A ----
            with tc.tile_pool(name="pA", bufs=2) as pA:
                xpad_sb = pA.tile([C, 66 * 66], BF, bufs=1)
                nc.sync.dma_start(xpad_sb[:], xpad[:])
                xpad3 = xpad_sb[:].rearrange("p (r q) -> p r q", q=66)
                xnc = pA.tile([C, L], BF, bufs=1)   # centered/normed (LN affine folded)
                xcf = pA.tile([C, L], BF, bufs=1)   # centered
                # pass0: pos-enc conv, dense PE burst (no tables)
                for c in range(NCH):
                    cs = slice(c * LC, (c + 1) * LC)
                    pa = psP.tile([128, 8, 64], F32, tag="gen", bufs=2)
                    for tap in range(9):
                        dy, dx = tap // 3, tap % 3
                        nc.tensor.matmul(
                            pa[:],
                            w9_sb[:, tap * 128:(tap + 1) * 128],
                            xpad3[:, c * 8 + dy:c * 8 + dy + 8, dx:dx + 64],
                            start=(tap == 0), stop=(tap == 8))
                    paf = pa[:].rearrange("p a b -> p (a b)")
                    pe_t = pA.tile([128, LC], BF, tag="pe")
                    nc.sync.dma_start(pe_t[:], pe_b[:, cs])
                    nc.vector.tensor_tensor(xs[:, cs], paf, pe_t[:], op=ALU.add)
                # pass1: LN in 1024-wide stages (tables: Sqrt; Square free)
                for c2 in range(NCH // 2):
                    c2s = slice(c2 * 2 * LC, (c2 + 1) * 2 * LC)
                    mu = psP.tile([1, 2 * LC], F32, tag="gen", bufs=2)
                    for half in range(2):
                        nc.tensor.matmul(
                            mu[:, half * LC:(half + 1) * LC], mred1_sb[:],
                            xs[:, (c2 * 2 + half) * LC:(c2 * 2 + half + 1) * LC],
                            start=True, stop=True)
                    mu_sb = pA.tile([1, 2 * LC], BF, tag="musb")
                    nc.scalar.copy(mu_sb[:], mu[:])
                    mub = psP.tile([128, 2 * LC], F32, tag="gen", bufs=2)
                    for half in range(2):
                        nc.tensor.matmul(
                            mub[:, half * LC:(half + 1) * LC], onesr_sb[:],
                            mu_sb[:, half * LC:(half + 1) * LC],
                            start=True, stop=True)
                    nc.vector.tensor_tensor(xcf[:, c2s], xs[:, c2s], mub[:],
                                            op=ALU.subtract)
                    xsq = pA.tile([128, 2 * LC], BF, tag="xsq")
                    nc.vector.tensor_tensor(xsq[:], xcf[:, c2s], xcf[:, c2s],
                                            op=ALU.mult)
                    var = psP.tile([1, 2 * LC], F32, tag="gen", bufs=2)
                    for half in range(2):
                        nc.tensor.matmul(
                            var[:, half * LC:(half + 1) * LC], mred1_sb[:],
                            xsq[:, half * LC:(half + 1) * LC],
                            start=True, stop=True)
                    sd = pA.tile([1, 2 * LC], F32, tag="sd")
                    nc.scalar.activation(sd[:], var[:], AF.Sqrt,
                                         bias=eps_sb[:, 0:1])
                    rstd = pA.tile([1, 2 * LC], BF, tag="rstd")
                    with nc.allow_low_precision(reason="bf16 rstd; tol 2e-2"):
                        nc.vector.reciprocal(rstd[:], sd[:])
                    rstdb = psP.tile([128, 2 * LC], F32, tag="gen", bufs=2)
                    for half in range(2):
                        nc.tensor.matmul(
                            rstdb[:, half * LC:(half + 1) * LC], onesr_sb[:],
                            rstd[:, half * LC:(half + 1) * LC],
                            start=True, stop=True)
                    nc.vector.tensor_tensor(xnc[:, c2s], xcf[:, c2s], rstdb[:],
                                            op=ALU.mult)
                # pass2: gate (Sigmoid)
                for c in range(NCH):
                    cs = slice(c * LC, (c + 1) * LC)
                    gps = psP.tile([128, LC], F32, tag="gen", bufs=2)
                    nc.tensor.matmul(gps[:], gateWT_sb[:], xnc[:, cs],
                                     start=True, stop=True)
                    nc.scalar.activation(gate[:, cs], gps[:], AF.Sigmoid,
                                         bias=gateb_sb[:, 0:1])
                # pass3: xz both local groups packed (Silu; Copy free)
                for c in range(NCH):
                    cs = slice(c * LC, (c + 1) * LC)
                    up = psP.tile([128, LC], F32, tag="gen", bufs=2)
                    nc.tensor.matmul(up[:], winTu_sb[:], xnc[:, cs],
                                     start=True, stop=True)
                    nc.scalar.copy(u_pad[:, 3 + c * LC:3 + (c + 1) * LC], up[:])
                    zp = psP.tile([128, LC], F32, tag="gen", bufs=2)
                    nc.tensor.matmul(zp[:], winTz_sb[:], xnc[:, cs],
                                     start=True, stop=True)
                    nc.scalar.activation(zs[:, cs], zp[:], AF.Silu,
                                         bias=zb_sb[:, 0:1])

            # ---- Phase B ----
            # All four (gl,dr) front-ends first (one table set per pass),
            # then the four j-loops (Exp table once).
            with tc.tile_pool(name="pB", bufs=2) as wp:
                uc_all = [bp.tile([128, L], BF, name=f"uc{i4}")
                          for i4 in range(4)]
                dt_all = [bp.tile([128, L], BF, name=f"dtt{i4}")
                          for i4 in range(4)]
                # FE pass1: conv + silu (Silu table)
                for gl in range(2):
                    rows = slice(gl * 64, gl * 64 + 64)
                    for dr in range(2):
                        i4 = gl * 2 + dr
                        uc = uc_all[i4]
                        for cc in range(NCH // 2):
                            ucp = psP.tile([128, 2 * LC], F32, tag="gen", bufs=2)
                            for half in range(2):
                                c = cc * 2 + half
                                for k in range(DC):
                                    off = (c * LC + k) if dr == 0 else (3 + c * LC + k)
                                    nc.tensor.matmul(
                                        ucp[:, half * LC:(half + 1) * LC],
                                        conv4T_sb[rows,
                                                  (i4 * 4 + k) * 128:
                                                  (i4 * 4 + k + 1) * 128],
                                        u_pad[rows, off:off + LC],
                                        start=(k == 0), stop=(k == DC - 1))
                            nc.scalar.activation(
                                uc[:, cc * 2 * LC:(cc + 1) * 2 * LC], ucp[:],
                                AF.Silu, bias=convb_sb[:, i4:i4 + 1])
                # FE pass2: dt matmul + sigmoid (Sigmoid table)
                for i4 in range(4):
                    uc, dt_t = uc_all[i4], dt_all[i4]
                    for cc in range(NCH // 2):
                        c2s = slice(cc * 2 * LC, (cc + 1) * 2 * LC)
                        dtp = psP.tile([128, 2 * LC], F32, tag="gen", bufs=2)
                        for half in range(2):
                            c = cc * 2 + half
                            nc.tensor.matmul(
                                dtp[:, half * LC:(half + 1) * LC],
                                dtWT_sb[:, i4 * 128:(i4 + 1) * 128],
                                uc[0:DI, c * LC:(c + 1) * LC],
                                start=True, stop=True)
                        nc.scalar.activation(dt_t[:, c2s], dtp[:],
                                             AF.Sigmoid,
                                             bias=dtb_sb[:, i4:i4 + 1],
                                             scale=-1.0)
                # FE pass3: dt_t = ln(sigmoid(..)) = -softplus (Ln table)
                for i4 in range(4):
                    nc.scalar.activation(dt_all[i4][:], dt_all[i4][:], AF.Ln)

                # j-loops (Exp table; Copy free)
                for gl in range(2):
                    rows = slice(gl * 64, gl * 64 + 64)
                    for dr in range(2):
                        i4 = gl * 2 + dr
                        uc, dt_t = uc_all[i4], dt_all[i4]
                        dtuc = wp.tile([128, L], BF, tag="dtuc", bufs=2)
                        nc.vector.tensor_tensor(dtuc[:], dt_t[:], uc[:],
                                                op=ALU.mult)
                        horder = (0, 1) if dr == 0 else (1, 0)
                        h_prev = [None] * NJ
                        for hf in horder:
                            hs = slice(hf * LH, (hf + 1) * LH)
                            first = (hf == horder[0])
                            ys = [psP.tile([128, LC], F32, tag=f"ys{q}",
                                           bufs=1, name=f"ys{q}")
                                  for q in range(NCC)]
                            prev_prod = None

                            def bcast(side, j):
                                # B/C broadcast for s-pair j, direct from uc
                                bb = wp.tile([128, LH], BF, bufs=3,
                                             tag=("bbB" if side == 0 else "bbC"))
                                wsl = ((i4 * NJ + j) * 2 + side) * 128
                                for p2 in range(2):
                                    bps = psP.tile([128, 2 * LC], F32,
                                                   tag="gen", bufs=2)
                                    for half in range(2):
                                        q = p2 * 2 + half
                                        nc.tensor.matmul(
                                            bps[:, half * LC:(half + 1) * LC],
                                            xprojJ_sb[:, wsl:wsl + 128],
                                            uc[0:DI, hf * LH + q * LC:
                                               hf * LH + (q + 1) * LC],
                                            start=True, stop=True)
                                    nc.scalar.copy(
                                        bb[:, p2 * 2 * LC:(p2 + 1) * 2 * LC],
                                        bps[:])
                                return bb

                            for j in range(NJ):
                                dA = wp.tile([128, LH], BF, tag="dA", bufs=2)
                                nc.scalar.activation(
                                    dA[:], dt_t[:, hs], AF.Exp,
                                    scale=acol_sb[:, i4 * NJ + j:i4 * NJ + j + 1])
                                bbB = bcast(0, j)
                                dBu = wp.tile([128, LH], BF, tag="dBu")
                                nc.vector.tensor_tensor(dBu[:], dtuc[:, hs],
                                                        bbB[:], op=ALU.mult)
                                h = hp.tile([128, LH], BF, tag="h")
                                hc = hp.tile([128, 1], BF, tag=f"hc{j}",
                                             name=f"hc{j}")
                                init = 0.0 if first else h_prev[j][:, 0:1]
                                if dr == 0:
                                    nc.vector.tensor_tensor_scan(
                                        h[:], dA[:], dBu[:], init,
                                        op0=ALU.mult, op1=ALU.add)
                                    nc.gpsimd.tensor_copy(hc[:], h[:, LH - 1:LH])
                                else:
                                    nc.vector.tensor_tensor_scan(
                                        h[:, ::-1], dA[:, ::-1], dBu[:, ::-1],
                                        init, op0=ALU.mult, op1=ALU.add)
                                    nc.gpsimd.tensor_copy(hc[:], h[:, 0:1])
                                h_prev[j] = hc
                                bbC = bcast(1, j)
                                prod = wp.tile([128, LH], BF, tag="prod")
                                nc.vector.tensor_tensor(prod[:], h[:], bbC[:],
                                                        op=ALU.mult)
                                if prev_prod is not None:
                                    for q in range(NCC):
                                        nc.tensor.matmul(
                                            ys[q][rows, :], mredM_sb[:, 0:DI],
                                            prev_prod[:, q * LC:(q + 1) * LC],
                                            start=(j - 1 == 0), stop=False)
                                prev_prod = prod
                            for q in range(NCC):
                                nc.tensor.matmul(
                                    ys[q][rows, :], mredM_sb[:, 0:DI],
                                    prev_prod[:, q * LC:(q + 1) * LC],
                                    start=(NJ == 1), stop=True)
                            # epilogue for this half
                            for q in range(NCC):
                                c = hf * NCC + q
                                cs = slice(c * LC, (c + 1) * LC)
                                y1 = wp.tile([128, LC], BF, tag="y1")
                                nc.vector.scalar_tensor_tensor(
                                    y1[rows, :], uc[rows, cs],
                                    dsk_sb[rows, i4:i4 + 1],
                                    ys[q][rows, :], op0=ALU.mult,
                                    op1=ALU.subtract)
                                if dr == 0:
                                    nc.gpsimd.tensor_tensor(yfb[rows, cs],
                                                            y1[rows, :],
                                                            zs[rows, cs],
                                                            op=ALU.mult)
                                else:
                                    y2 = wp.tile([128, LC], BF, tag="y2")
                                    nc.gpsimd.tensor_tensor(y2[rows, :],
                                                            y1[rows, :],
                                                            zs[rows, cs],
                                                            op=ALU.mult)
                                    nc.gpsimd.tensor_tensor(yfb[rows, cs],
                                                            yfb[rows, cs],
                                                            y2[rows, :],
                                                            op=ALU.add)

            # ---- Phase C: Wout, exchange, blend, proj ----
            with tc.tile_pool(name="pC", bufs=2) as wpc:
                for c in range(NCH):
                    cs = slice(c * LC, (c + 1) * LC)
                    ymp = psP.tile([2 * DM, LC], F32, tag="gen", bufs=2)
                    nc.tensor.matmul(ymp[:], woutT_sb[:], yfb[:, cs],
                                     start=True, stop=True)
                    ym_sb = wpc.tile([2 * DM, LC], BF, tag="ymsb")
                    nc.scalar.copy(ym_sb[:], ymp[:])
                    nc.sync.dma_start(ym_loc[:, cs], ym_sb[:])
                nc.gpsimd.collective_compute(
                    "AllGather", ALU.bypass,
                    replica_groups=[[0, 1], [2, 3], [4, 5], [6, 7]],
                    ins=[ym_loc[:]], outs=[ym_all[:]])
                for c in range(NCH):
                    cs = slice(c * LC, (c + 1) * LC)
                    xm_t = wpc.tile([C, LC], BF, tag="xmt")
                    nc.sync.dma_start(xm_t[:], ym_all[:, cs])
                    ta = wpc.tile([128, LC], BF, tag="ta")
                    nc.gpsimd.tensor_tensor(ta[:], xm_t[:], xs[:, cs],
                                            op=ALU.subtract)
                    tb2 = wpc.tile([128, LC], BF, tag="tb")
                    nc.vector.tensor_tensor(tb2[:], gate[:, cs], ta[:],
                                            op=ALU.mult)
                    tc2 = wpc.tile([128, LC], BF, tag="tc")
                    nc.gpsimd.tensor_tensor(tc2[:], xs[:, cs], tb2[:],
                                            op=ALU.add)
                    op_ = psP.tile([128, LC], F32, tag="gen", bufs=2)
                    nc.tensor.matmul(op_[:], projT_sb[:], tc2[:],
                                     start=True, stop=True)
                    osb = wpc.tile([128, LC], F32, tag="osb")
                    nc.scalar.activation(osb[:], op_[:], AF.Identity,
                                         bias=projb_sb[:, 0:1])
                    nc.sync.dma_start(outp[:, cs], osb[:])
    nc.finalize()
    return nc


def _bf(a):
    import concourse.mybir as _mb
    return np.asarray(a).astype(_mb.dt.np(_mb.dt.bfloat16))


def _prep_inputs(inputs):
    """Build the 8 per-core in_maps from full inputs."""
    ii = {k: np.asarray(v, dtype=np.float32) for k, v in inputs.items()}
    x = ii["x"]

    maps_w = []  # weight dicts per group-set gs=0,1
    for gs in range(2):
        w = {}
        w9 = np.zeros((C, 9 * 128), np.float32)
        for tap in range(9):
            dy, dx = tap // 3, tap % 3
            blk = np.zeros((C, 128), np.float32)
            np.fill_diagonal(blk, ii["pos_conv_w"][:, 0, dy, dx])
            if tap == 4:
                blk[np.arange(C), np.arange(C)] += 1.0
            w9[:, tap * 128:(tap + 1) * 128] = blk
        w["w9"] = _bf(w9)
        w["pe_b"] = _bf(np.ascontiguousarray(ii["pos_embed"][0].T)
                        + ii["pos_conv_b"][:, None])
        w["mred1"] = _bf(np.full((128, 1), 1.0 / 128, np.float32))
        w["onesr"] = _bf(np.ones((1, 128), np.float32))
        lng = ii["ln_g"]
        lnb = ii["ln_b"]
        # LN affine folded into consumers: xnc on-device is (x-mu)/sd
        w["gateWT"] = _bf(ii["gate_W"].T * lng[:, None])
        w["gateb"] = np.ascontiguousarray(
            (ii["gate_b"] + ii["gate_W"] @ lnb)[:, None])
        w["projT"] = _bf(ii["proj_W"].T)
        w["projb"] = np.ascontiguousarray(ii["proj_b"][:, None])
        w["mredM"] = _bf(np.tile(np.eye(DI, dtype=np.float32), (2, 1)))
        winTu = np.zeros((C, 128), np.float32)
        winTz = np.zeros((C, 128), np.float32)
        zb = np.zeros((128, 1), np.float32)
        ub = np.zeros((128,), np.float32)
        conv4T = np.zeros((2, 2, DC, DI, 128), np.float32)
        convb = np.zeros((2, 2, 128, 1), np.float32)
        dtWT = np.zeros((2, 2, DI, 128), np.float32)
        dtb = np.zeros((2, 2, 128, 1), np.float32)
        xprojJ = np.zeros((2, 2, NJ, 2, DI, 128), np.float32)
        A_col = np.zeros((2, 2, 128, NJ), np.float32)
        dsk = np.zeros((2, 2, 128, 1), np.float32)
        woutT = np.zeros((128, 2 * DM), np.float32)
        for gl in range(2):
            gg = gs * 2 + gl
            gsl = slice(gg * DM, (gg + 1) * DM)
            Wu = ii["m_Win"][gg, 0:DI, :]        # (DI, DM)
            Wz = ii["m_Win"][gg, DI:2 * DI, :]
            winTu[gsl, gl * DI:(gl + 1) * DI] = (Wu * lng[None, gsl]).T
            winTz[gsl, gl * DI:(gl + 1) * DI] = (Wz * lng[None, gsl]).T
            ub[gl * DI:(gl + 1) * DI] = Wu @ lnb[gsl]
            zb[gl * DI:(gl + 1) * DI, 0] = Wz @ lnb[gsl]
            woutT[gl * 64:(gl + 1) * 64, gl * DM:(gl + 1) * DM] = ii["m_Wout"][gg].T
            for dr in range(2):
                for k in range(DC):
                    wk = ii["conv_w"][gg, dr, :, k if dr == 0 else DC - 1 - k]
                    blk = np.zeros((DI, 128), np.float32)
                    blk[np.arange(DI), np.arange(DI)] = wk
                    blk[np.arange(DI), 64 + np.arange(DI)] = wk
                    conv4T[gl, dr, k] = blk
                convb[gl, dr, :, 0] = np.tile(
                    ii["conv_b"][gg, dr]
                    + ii["conv_w"][gg, dr].sum(-1) * ub[gl * DI:(gl + 1) * DI], 2)
                M2 = ii["dt_W"][gg, dr] @ ii["xproj_W"][gg, dr][0:DTR, :]
                dtWT[gl, dr] = np.concatenate([M2.T, M2.T], axis=1)
                dtb[gl, dr, :, 0] = -np.tile(ii["dt_b"][gg, dr], 2)
                Wb = ii["xproj_W"][gg, dr][DTR:DTR + DS, :]        # (DS, DI)
                Wc = ii["xproj_W"][gg, dr][DTR + DS:DTR + 2 * DS, :]
                p = np.arange(128)
                for j in range(NJ):
                    xprojJ[gl, dr, j, 0] = Wb[2 * j + p[None, :] // 64,
                                              np.arange(DI)[:, None]]
                    xprojJ[gl, dr, j, 1] = Wc[2 * j + p[None, :] // 64,
                                              np.arange(DI)[:, None]]
                A = np.exp(ii["A_log"][gg, dr])  # (DI, DS); dt_t is -dt
                for j in range(NJ):
                    A_col[gl, dr, :, j] = A[p % 64, 2 * j + p // 64]
                dsk[gl, dr, :, 0] = np.tile(ii["Dskip"][gg, dr], 2)
        w.update(winTu=_bf(winTu), winTz=_bf(winTz), zb=zb,
                 ub_neg=_bf(np.tile(-ub[:, None], (1, 3))),
                 conv4T=_bf(conv4T), convb=convb, dtWT=_bf(dtWT), dtb=dtb,
                 xprojJ=_bf(xprojJ), A_col=A_col, dsk=dsk, woutT=_bf(woutT))
        maps_w.append(w)

    in_maps = []
    for k in range(NCORE):
        b, gs = k // 2, k % 2
        m = dict(maps_w[gs])
        xp = np.zeros((C, 66, 66), np.float32)
        xp[:, 1:65, 1:65] = x[b]
        m["xpad"] = _bf(np.ascontiguousarray(xp.reshape(C, 66 * 66)))
        in_maps.append(m)
    return in_maps


_CACHE = {}


def kernel(**inputs):
    from concourse.bass_utils import run_bass_kernel_spmd
    if "nc" not in _CACHE:
        _CACHE["nc"] = _build_nc()
    nc = _CACHE["nc"]
    in_maps = _prep_inputs(inputs)
    res = run_bass_kernel_spmd(nc, in_maps, list(range(NCORE))).results
    out = np.stack([np.asarray(res[2 * b]["outp"]).reshape(OUT, H, W)
                    for b in range(B)])
    return out.astype(np.float32)


# revision 46
# speedup vs baseline: 1.4751x; 1.0645x over previous
"""Trainium2 Bass kernel for CDMamba ModifiedSRCMLayer (self-contained).

Sharding: 8 cores; core k handles batch k//2 and mamba group-pair k%2
(groups {0,1} or {2,3}). Group outputs are exchanged with a paired
AllGather; the post-stage (gate blend + output projection) is computed
redundantly on both cores of a pair and the host reads even cores.

v2: all-bf16 datapath. Phase A/C in bf16 packed matmuls, grouped into
activation-table passes (Sqrt | Sigmoid | Silu). Phase B front-end uses
AF.Silu / AF.Softplus directly; the j-loop broadcasts B/C with PE
matmuls, copies PSUM->SBUF bf16 on the ACT engine, and runs dBu/prod as
[128,2048] pure-bf16 DVE tensor_tensor ops (16-bit 2x rate) plus the
DVE tensor_tensor_scan. The s-reduction accumulates in PSUM via matmul.
"""
import sys
import numpy as np

for _p in ("/opt/trn_rl_repo",):
    if _p not in sys.path:
        sys.path.append(_p)

import concourse.bass as bass
import concourse.mybir as mybir
from concourse.bacc import Bacc
from concourse.tile import TileContext

# Model dims (hardcoded per the problem spec)
B, C, H, W = 4, 128, 64, 64
L = H * W                      # 4096
G, DM = 4, 32
DI, DS, DC = 64, 16, 4
DTR = 2
OUT = 128
EPS = 1e-5

NCORE = 8
LC = 512
NCH = L // LC                  # 8
LH = L // 2                    # 2048
NCC = LH // LC                 # 4
NJ = DS // 2                   # 8 j-tiles (2 s-values per tile)
F32 = mybir.dt.float32
BF = mybir.dt.bfloat16
AF = mybir.ActivationFunctionType
ALU = mybir.AluOpType


def _build_nc():
    nc = Bacc(num_devices=NCORE)

    def inp(name, shape, dt=BF):
        return nc.dram_tensor(name, list(shape), dt, kind="ExternalInput")

    xpad = inp("xpad", (C, 66 * 66))
    pe_b = inp("pe_b", (C, L))
    w9 = inp("w9", (C, 9 * 128))
    mred1 = inp("mred1", (128, 1))
    onesr = inp("onesr", (1, 128))
    gateWT = inp("gateWT", (128, 128))
    gateb = inp("gateb", (128, 1), F32)
    winTu = inp("winTu", (C, 128))       # u for both local groups
    winTz = inp("winTz", (C, 128))
    zb = inp("zb", (128, 1), F32)        # ln-affine fold: Win_z @ ln_b
    ub_neg = inp("ub_neg", (128, 3))     # -Win_u @ ln_b (conv halo)
    # pre-packed in SBUF layout (single DMA each)
    conv4T = inp("conv4T", (128, 16 * 128))
    convb = inp("convb", (128, 4), F32)
    dtWT = inp("dtWT", (DI, 4 * 128))
    dtb = inp("dtb", (128, 4), F32)
    xprojJ = inp("xprojJ", (DI, 4 * NJ * 2 * 128))  # replicated B/C weights
    A_col = inp("A_col", (128, 4 * NJ), F32)
    dsk = inp("dsk", (128, 4), F32)
    mredM = inp("mredM", (128, DI))
    woutT = inp("woutT", (128, 2 * DM))
    projT = inp("projT", (128, 128))
    projb = inp("projb", (128, 1), F32)

    ym_loc = nc.dram_tensor("ym_loc", [2 * DM, L], BF)
    ym_all = nc.dram_tensor("ym_all", [C, L], BF)
    outp = nc.dram_tensor("outp", [OUT, L], F32, kind="ExternalOutput")

    with TileContext(nc) as tc:
        with (
            tc.tile_pool(name="const", bufs=1) as cp,
            tc.tile_pool(name="big", bufs=1) as bp,
            tc.tile_pool(name="hpool", bufs=2) as hp,
            tc.tile_pool(name="psP", bufs=1, space="PSUM") as psP,
        ):
            # ---- constants to SBUF ----
            def c_load(ap_dram, shape, nm, dt=BF):
                t = cp.tile(list(shape), dt, name=nm, tag=nm)
                nc.sync.dma_start(t[:], ap_dram)
                return t

            # w9/xpad first (first conv depends on them), split across queues
            w9_sb = cp.tile([C, 9 * 128], BF, name="w9sb", tag="w9sb")
            for qq in range(4):
                eng = (nc.sync, nc.scalar, nc.gpsimd, nc.scalar)[qq]
                eng.dma_start(w9_sb[:, qq * 288:(qq + 1) * 288],
                              w9[:, qq * 288:(qq + 1) * 288])
            mred1_sb = c_load(mred1[:], (128, 1), "mred1sb")
            onesr_sb = c_load(onesr[:], (1, 128), "onesrsb")
            gateWT_sb = c_load(gateWT[:], (128, 128), "gateWTsb")
            gateb_sb = c_load(gateb[:], (128, 1), "gatebsb", F32)
            winTu_sb = c_load(winTu[:], (C, 128), "winTusb")
            winTz_sb = c_load(winTz[:], (C, 128), "winTzsb")
            zb_sb = c_load(zb[:], (128, 1), "zbsb", F32)
            ubn_sb = c_load(ub_neg[:], (128, 3), "ubnsb")
            mredM_sb = c_load(mredM[:], (128, DI), "mredMsb")
            woutT_sb = c_load(woutT[:], (128, 2 * DM), "woutTsb")
            projT_sb = c_load(projT[:], (128, 128), "projTsb")
            projb_sb = c_load(projb[:], (128, 1), "projbsb", F32)

            conv4T_sb = cp.tile([128, 16 * 128], BF)
            nc.scalar.dma_start(conv4T_sb[:], conv4T[:])
            dtWT_sb = cp.tile([DI, 4 * 128], BF)
            nc.gpsimd.dma_start(dtWT_sb[:], dtWT[:])
            xprojJ_sb = cp.tile([DI, 4 * NJ * 2 * 128], BF)
            for qq in range(2):
                eng = (nc.scalar, nc.gpsimd)[qq]
                eng.dma_start(xprojJ_sb[:, qq * 4096:(qq + 1) * 4096],
                              xprojJ[:, qq * 4096:(qq + 1) * 4096])
            acol_sb = cp.tile([128, 4 * NJ], F32)
            nc.sync.dma_start(acol_sb[:], A_col[:])
            convb_sb = cp.tile([128, 4], F32)
            nc.sync.dma_start(convb_sb[:], convb[:])
            dtb_sb = cp.tile([128, 4], F32)
            nc.sync.dma_start(dtb_sb[:], dtb[:])
            dsk_sb = cp.tile([128, 4], F32)
            nc.sync.dma_start(dsk_sb[:], dsk[:])
            eps_sb = cp.tile([1, 1], F32)
            nc.vector.memset(eps_sb[:], EPS)

            # ---- persistent tiles ----
            xs = bp.tile([C, L], BF)
            gate = bp.tile([C, L], BF)
            u_pad = bp.tile([C, L + 6], BF)
            zs = bp.tile([C, L], BF)
            yfb = bp.tile([C, L], BF)

            # halo = -Win_u@ln_b so the folded-LN conv matches zero-padded ref
            nc.vector.tensor_copy(u_pad[:, 0:3], ubn_sb[:])
            nc.vector.tensor_copy(u_pad[:, L + 3:L + 6], ubn_sb[:])

            # ---- Phase A ----
            with tc.tile_pool(name="pA", bufs=2) as pA:
                xpad_sb = pA.tile([C, 66 * 66], BF, bufs=1)
                for qq in range(4):
                    eng = (nc.sync, nc.scalar, nc.gpsimd, nc.scalar)[qq]
                    eng.dma_start(xpad_sb[:, qq * 1089:(qq + 1) * 1089],
                                  xpad[:, qq * 1089:(qq + 1) * 1089])
                xpad3 = xpad_sb[:].rearrange("p (r q) -> p r q", q=66)
                xnc = pA.tile([C, L], BF, bufs=1)   # centered/normed (LN affine folded)
                xcf = pA.tile([C, L], BF, bufs=1)   # centered
                # pass0: pos-enc conv, dense PE burst (no tables)
                for c in range(NCH):
                    cs = slice(c * LC, (c + 1) * LC)
                    pa = psP.tile([128, 8, 64], F32, tag="gen", bufs=2)
                    for tap in range(9):
                        dy, dx = tap // 3, tap % 3
                        nc.tensor.matmul(
                            pa[:],
                            w9_sb[:, tap * 128:(tap + 1) * 128],
                            xpad3[:, c * 8 + dy:c * 8 + dy + 8, dx:dx + 64],
                            start=(tap == 0), stop=(tap == 8))
                    paf = pa[:].rearrange("p a b -> p (a b)")
                    pe_t = pA.tile([128, LC], BF, tag="pe", bufs=3)
                    eng = (nc.scalar, nc.sync, nc.gpsimd)[c % 3]
                    eng.dma_start(pe_t[:], pe_b[:, cs])
                    nc.vector.tensor_tensor(xs[:, cs], paf, pe_t[:], op=ALU.add)
                # pass1: LN in 1024-wide stages (tables: Sqrt; Square free)
                for c2 in range(NCH // 2):
                    c2s = slice(c2 * 2 * LC, (c2 + 1) * 2 * LC)
                    mu = psP.tile([1, 2 * LC], F32, tag="gen", bufs=2)
                    for half in range(2):
                        nc.tensor.matmul(
                            mu[:, half * LC:(half + 1) * LC], mred1_sb[:],
                            xs[:, (c2 * 2 + half) * LC:(c2 * 2 + half + 1) * LC],
                            start=True, stop=True)
                    mu_sb = pA.tile([1, 2 * LC], BF, tag="musb")
                    nc.scalar.copy(mu_sb[:], mu[:])
                    mub = psP.tile([128, 2 * LC], F32, tag="gen", bufs=2)
                    for half in range(2):
                        nc.tensor.matmul(
                            mub[:, half * LC:(half + 1) * LC], onesr_sb[:],
                            mu_sb[:, half * LC:(half + 1) * LC],
                            start=True, stop=True)
                    nc.vector.tensor_tensor(xcf[:, c2s], xs[:, c2s], mub[:],
                                            op=ALU.subtract)
                    xsq = pA.tile([128, 2 * LC], BF, tag="xsq")
                    nc.vector.tensor_tensor(xsq[:], xcf[:, c2s], xcf[:, c2s],
                                            op=ALU.mult)
                    var = psP.tile([1, 2 * LC], F32, tag="gen", bufs=2)
                    for half in range(2):
                        nc.tensor.matmul(
                            var[:, half * LC:(half + 1) * LC], mred1_sb[:],
                            xsq[:, half * LC:(half + 1) * LC],
                            start=True, stop=True)
                    sd = pA.tile([1, 2 * LC], F32, tag="sd")
                    nc.scalar.activation(sd[:], var[:], AF.Sqrt,
                                         bias=eps_sb[:, 0:1])
                    rstd = pA.tile([1, 2 * LC], BF, tag="rstd")
                    with nc.allow_low_precision(reason="bf16 rstd; tol 2e-2"):
                        nc.vector.reciprocal(rstd[:], sd[:])
                    rstdb = psP.tile([128, 2 * LC], F32, tag="gen", bufs=2)
                    for half in range(2):
                        nc.tensor.matmul(
                            rstdb[:, half * LC:(half + 1) * LC], onesr_sb[:],
                            rstd[:, half * LC:(half + 1) * LC],
                            start=True, stop=True)
                    nc.vector.tensor_tensor(xnc[:, c2s], xcf[:, c2s], rstdb[:],
                                            op=ALU.mult)
                # pass2: gate (Sigmoid)
                for c in range(NCH):
                    cs = slice(c * LC, (c + 1) * LC)
                    gps = psP.tile([128, LC], F32, tag="gen", bufs=2)
                    nc.tensor.matmul(gps[:], gateWT_sb[:], xnc[:, cs],
                                     start=True, stop=True)
                    nc.scalar.activation(gate[:, cs], gps[:], AF.Sigmoid,
                                         bias=gateb_sb[:, 0:1])
                # pass3: xz both local groups packed (Silu; Copy free)
                for c in range(NCH):
                    cs = slice(c * LC, (c + 1) * LC)
                    up = psP.tile([128, LC], F32, tag="gen", bufs=2)
                    nc.tensor.matmul(up[:], winTu_sb[:], xnc[:, cs],
                                     start=True, stop=True)
                    nc.scalar.copy(u_pad[:, 3 + c * LC:3 + (c + 1) * LC], up[:])
                    zp = psP.tile([128, LC], F32, tag="gen", bufs=2)
                    nc.tensor.matmul(zp[:], winTz_sb[:], xnc[:, cs],
                                     start=True, stop=True)
                    nc.scalar.activation(zs[:, cs], zp[:], AF.Silu,
                                         bias=zb_sb[:, 0:1])

            # ---- Phase B ----
            # All four (gl,dr) front-ends first (one table set per pass),
            # then the four j-loops (Exp table once).
            with tc.tile_pool(name="pB", bufs=2) as wp:
                uc_all = [bp.tile([128, L], BF, name=f"uc{i4}")
                          for i4 in range(4)]
                dt_all = [bp.tile([128, L], BF, name=f"dtt{i4}")
                          for i4 in range(4)]
                # FE pass1: conv + silu (Silu table)
                for gl in range(2):
                    rows = slice(gl * 64, gl * 64 + 64)
                    for dr in range(2):
                        i4 = gl * 2 + dr
                        uc = uc_all[i4]
                        for cc in range(NCH // 2):
                            ucp = psP.tile([128, 2 * LC], F32, tag="gen", bufs=2)
                            for half in range(2):
                                c = cc * 2 + half
                                for k in range(DC):
                                    off = (c * LC + k) if dr == 0 else (3 + c * LC + k)
                                    nc.tensor.matmul(
                                        ucp[:, half * LC:(half + 1) * LC],
                                        conv4T_sb[rows,
                                                  (i4 * 4 + k) * 128:
                                                  (i4 * 4 + k + 1) * 128],
                                        u_pad[rows, off:off + LC],
                                        start=(k == 0), stop=(k == DC - 1))
                            nc.scalar.activation(
                                uc[:, cc * 2 * LC:(cc + 1) * 2 * LC], ucp[:],
                                AF.Silu, bias=convb_sb[:, i4:i4 + 1])
                # FE pass2: dt matmul + sigmoid (Sigmoid table)
                for i4 in range(4):
                    uc, dt_t = uc_all[i4], dt_all[i4]
                    for cc in range(NCH // 2):
                        c2s = slice(cc * 2 * LC, (cc + 1) * 2 * LC)
                        dtp = psP.tile([128, 2 * LC], F32, tag="gen", bufs=2)
                        for half in range(2):
                            c = cc * 2 + half
                            nc.tensor.matmul(
                                dtp[:, half * LC:(half + 1) * LC],
                                dtWT_sb[:, i4 * 128:(i4 + 1) * 128],
                                uc[0:DI, c * LC:(c + 1) * LC],
                                start=True, stop=True)
                        nc.scalar.activation(dt_t[:, c2s], dtp[:],
                                             AF.Sigmoid,
                                             bias=dtb_sb[:, i4:i4 + 1],
                                             scale=-1.0)
                # FE pass3: dt_t = ln(sigmoid(..)) = -softplus (Ln table)
                for i4 in range(4):
                    nc.scalar.activation(dt_all[i4][:], dt_all[i4][:], AF.Ln)

                # j-loops (Exp table; Copy free)
                for gl in range(2):
                    rows = slice(gl * 64, gl * 64 + 64)
                    for dr in range(2):
                        i4 = gl * 2 + dr
                        uc, dt_t = uc_all[i4], dt_all[i4]
                        dtuc = wp.tile([128, L], BF, tag="dtuc", bufs=2)
                        nc.vector.tensor_tensor(dtuc[:], dt_t[:], uc[:],
                                                op=ALU.mult)
                        horder = (0, 1) if dr == 0 else (1, 0)
                        h_prev = [None] * NJ
                        for hf in horder:
                            hs = slice(hf * LH, (hf + 1) * LH)
                            first = (hf == horder[0])
                            ys = [psP.tile([128, LC], F32, tag=f"ys{q}",
                                           bufs=1, name=f"ys{q}")
                                  for q in range(NCC)]
                            prev_prod = None

                            def bcast(side, j):
                                # B/C broadcast for s-pair j, direct from uc
                                bb = wp.tile([128, LH], BF, bufs=3,
                                             tag=("bbB" if side == 0 else "bbC"))
                                wsl = ((i4 * NJ + j) * 2 + side) * 128
                                for p2 in range(2):
                                    bps = psP.tile([128, 2 * LC], F32,
                                                   tag="gen", bufs=2)
                                    for half in range(2):
                                        q = p2 * 2 + half
                                        nc.tensor.matmul(
                                            bps[:, half * LC:(half + 1) * LC],
                                            xprojJ_sb[:, wsl:wsl + 128],
                                            uc[0:DI, hf * LH + q * LC:
                                               hf * LH + (q + 1) * LC],
                                            start=True, stop=True)
                                    nc.scalar.copy(
                                        bb[:, p2 * 2 * LC:(p2 + 1) * 2 * LC],
                                        bps[:])
                                return bb

                            for j in range(NJ):
                                dA = wp.tile([128, LH], BF, tag="dA", bufs=2)
                                nc.scalar.activation(
                                    dA[:], dt_t[:, hs], AF.Exp,
                                    scale=acol_sb[:, i4 * NJ + j:i4 * NJ + j + 1])
                                bbB = bcast(0, j)
                                dBu = wp.tile([128, LH], BF, tag="dBu")
                                nc.vector.tensor_tensor(dBu[:], dtuc[:, hs],
                                                        bbB[:], op=ALU.mult)
                                h = hp.tile([128, LH], BF, tag="h")
                                hc = hp.tile([128, 1], BF, tag=f"hc{j}",
                                             name=f"hc{j}")
                                init = 0.0 if first else h_prev[j][:, 0:1]
                                if dr == 0:
                                    nc.vector.tensor_tensor_scan(
                                        h[:], dA[:], dBu[:], init,
                                        op0=ALU.mult, op1=ALU.add)
                                    nc.gpsimd.tensor_copy(hc[:], h[:, LH - 1:LH])
                                else:
                                    nc.vector.tensor_tensor_scan(
                                        h[:, ::-1], dA[:, ::-1], dBu[:, ::-1],
                                        init, op0=ALU.mult, op1=ALU.add)
                                    nc.gpsimd.tensor_copy(hc[:], h[:, 0:1])
                                h_prev[j] = hc
                                bbC = bcast(1, j)
                                prod = wp.tile([128, LH], BF, tag="prod")
                                nc.vector.tensor_tensor(prod[:], h[:], bbC[:],
                                                        op=ALU.mult)
                                if prev_prod is not None:
                                    for q in range(NCC):
                                        nc.tensor.matmul(
                                            ys[q][rows, :], mredM_sb[:, 0:DI],
                                            prev_prod[:, q * LC:(q + 1) * LC],
                                            start=(j - 1 == 0), stop=False)
                                prev_prod = prod
                            for q in range(NCC):
                                nc.tensor.matmul(
                                    ys[q][rows, :], mredM_sb[:, 0:DI],
                                    prev_prod[:, q * LC:(q + 1) * LC],
                                    start=(NJ == 1), stop=True)
                            # epilogue for this half
                            for q in range(NCC):
                                c = hf * NCC + q
                                cs = slice(c * LC, (c + 1) * LC)
                                y1 = wp.tile([128, LC], BF, tag="y1")
                                nc.vector.scalar_tensor_tensor(
                                    y1[rows, :], uc[rows, cs],
                                    dsk_sb[rows, i4:i4 + 1],
                                    ys[q][rows, :], op0=ALU.mult,
                                    op1=ALU.subtract)
                                if dr == 0:
                                    nc.gpsimd.tensor_tensor(yfb[rows, cs],
                                                            y1[rows, :],
                                                            zs[rows, cs],
                                                            op=ALU.mult)
                                else:
                                    y2 = wp.tile([128, LC], BF, tag="y2")
                                    nc.gpsimd.tensor_tensor(y2[rows, :],
                                                            y1[rows, :],
                                                            zs[rows, cs],
                                                            op=ALU.mult)
                                    nc.gpsimd.tensor_tensor(yfb[rows, cs],
                                                            yfb[rows, cs],
                                                            y2[rows, :],
                                                            op=ALU.add)

            # ---- Phase C: Wout, exchange, blend, proj ----
            with tc.tile_pool(name="pC", bufs=2) as wpc:
                for c in range(NCH):
                    cs = slice(c * LC, (c + 1) * LC)
                    ymp = psP.tile([2 * DM, LC], F32, tag="gen", bufs=2)
                    nc.tensor.matmul(ymp[:], woutT_sb[:], yfb[:, cs],
                                     start=True, stop=True)
                    ym_sb = wpc.tile([2 * DM, LC], BF, tag="ymsb")
                    nc.scalar.copy(ym_sb[:], ymp[:])
                    eng = (nc.sync, nc.scalar, nc.gpsimd, nc.sync)[c % 4]
                    eng.dma_start(ym_loc[:, cs], ym_sb[:])
                nc.gpsimd.collective_compute(
                    "AllGather", ALU.bypass,
                    replica_groups=[[0, 1], [2, 3], [4, 5], [6, 7]],
                    ins=[ym_loc[:]], outs=[ym_all[:]])
                for c in range(NCH):
                    cs = slice(c * LC, (c + 1) * LC)
                    xm_t = wpc.tile([C, LC], BF, tag="xmt", bufs=3)
                    eng = (nc.sync, nc.scalar, nc.gpsimd, nc.sync)[c % 4]
                    eng.dma_start(xm_t[:], ym_all[:, cs])
                    ta = wpc.tile([128, LC], BF, tag="ta")
                    nc.gpsimd.tensor_tensor(ta[:], xm_t[:], xs[:, cs],
                                            op=ALU.subtract)
                    tb2 = wpc.tile([128, LC], BF, tag="tb")
                    nc.vector.tensor_tensor(tb2[:], gate[:, cs], ta[:],
                                            op=ALU.mult)
                    tc2 = wpc.tile([128, LC], BF, tag="tc")
                    nc.gpsimd.tensor_tensor(tc2[:], xs[:, cs], tb2[:],
                                            op=ALU.add)
                    op_ = psP.tile([128, LC], F32, tag="gen", bufs=2)
                    nc.tensor.matmul(op_[:], projT_sb[:], tc2[:],
                                     start=True, stop=True)
                    osb = wpc.tile([128, LC], F32, tag="osb", bufs=3)
                    nc.scalar.activation(osb[:], op_[:], AF.Identity,
                                         bias=projb_sb[:, 0:1])
                    eng = (nc.sync, nc.gpsimd, nc.scalar, nc.sync)[c % 4]
                    eng.dma_start(outp[:, cs], osb[:])
    nc.finalize()
    return nc


def _bf(a):
    import concourse.mybir as _mb
    return np.asarray(a).astype(_mb.dt.np(_mb.dt.bfloat16))


def _prep_inputs(inputs):
    """Build the 8 per-core in_maps from full inputs."""
    ii = {k: np.asarray(v, dtype=np.float32) for k, v in inputs.items()}
    x = ii["x"]

    maps_w = []  # weight dicts per group-set gs=0,1
    for gs in range(2):
        w = {}
        w9 = np.zeros((C, 9 * 128), np.float32)
        for tap in range(9):
            dy, dx = tap // 3, tap % 3
            blk = np.zeros((C, 128), np.float32)
            np.fill_diagonal(blk, ii["pos_conv_w"][:, 0, dy, dx])
            if tap == 4:
                blk[np.arange(C), np.arange(C)] += 1.0
            w9[:, tap * 128:(tap + 1) * 128] = blk
        w["w9"] = _bf(w9)
        w["pe_b"] = _bf(np.ascontiguousarray(ii["pos_embed"][0].T)
                        + ii["pos_conv_b"][:, None])
        w["mred1"] = _bf(np.full((128, 1), 1.0 / 128, np.float32))
        w["onesr"] = _bf(np.ones((1, 128), np.float32))
        lng = ii["ln_g"]
        lnb = ii["ln_b"]
        # LN affine folded into consumers: xnc on-device is (x-mu)/sd
        w["gateWT"] = _bf(ii["gate_W"].T * lng[:, None])
        w["gateb"] = np.ascontiguousarray(
            (ii["gate_b"] + ii["gate_W"] @ lnb)[:, None])
        w["projT"] = _bf(ii["proj_W"].T)
        w["projb"] = np.ascontiguousarray(ii["proj_b"][:, None])
        w["mredM"] = _bf(np.tile(np.eye(DI, dtype=np.float32), (2, 1)))
        winTu = np.zeros((C, 128), np.float32)
        winTz = np.zeros((C, 128), np.float32)
        zb = np.zeros((128, 1), np.float32)
        ub = np.zeros((128,), np.float32)
        conv4T = np.zeros((2, 2, DC, DI, 128), np.float32)
        convb = np.zeros((2, 2, 128, 1), np.float32)
        dtWT = np.zeros((2, 2, DI, 128), np.float32)
        dtb = np.zeros((2, 2, 128, 1), np.float32)
        xprojJ = np.zeros((2, 2, NJ, 2, DI, 128), np.float32)
        A_col = np.zeros((2, 2, 128, NJ), np.float32)
        dsk = np.zeros((2, 2, 128, 1), np.float32)
        woutT = np.zeros((128, 2 * DM), np.float32)
        for gl in range(2):
            gg = gs * 2 + gl
            gsl = slice(gg * DM, (gg + 1) * DM)
            Wu = ii["m_Win"][gg, 0:DI, :]        # (DI, DM)
            Wz = ii["m_Win"][gg, DI:2 * DI, :]
            winTu[gsl, gl * DI:(gl + 1) * DI] = (Wu * lng[None, gsl]).T
            winTz[gsl, gl * DI:(gl + 1) * DI] = (Wz * lng[None, gsl]).T
            ub[gl * DI:(gl + 1) * DI] = Wu @ lnb[gsl]
            zb[gl * DI:(gl + 1) * DI, 0] = Wz @ lnb[gsl]
            woutT[gl * 64:(gl + 1) * 64, gl * DM:(gl + 1) * DM] = ii["m_Wout"][gg].T
            for dr in range(2):
                for k in range(DC):
                    wk = ii["conv_w"][gg, dr, :, k if dr == 0 else DC - 1 - k]
                    blk = np.zeros((DI, 128), np.float32)
                    blk[np.arange(DI), np.arange(DI)] = wk
                    blk[np.arange(DI), 64 + np.arange(DI)] = wk
                    conv4T[gl, dr, k] = blk
                convb[gl, dr, :, 0] = np.tile(
                    ii["conv_b"][gg, dr]
                    + ii["conv_w"][gg, dr].sum(-1) * ub[gl * DI:(gl + 1) * DI], 2)
                M2 = ii["dt_W"][gg, dr] @ ii["xproj_W"][gg, dr][0:DTR, :]
                dtWT[gl, dr] = np.concatenate([M2.T, M2.T], axis=1)
                dtb[gl, dr, :, 0] = -np.tile(ii["dt_b"][gg, dr], 2)
                Wb = ii["xproj_W"][gg, dr][DTR:DTR + DS, :]        # (DS, DI)
                Wc = ii["xproj_W"][gg, dr][DTR + DS:DTR + 2 * DS, :]
                p = np.arange(128)
                for j in range(NJ):
                    xprojJ[gl, dr, j, 0] = Wb[2 * j + p[None, :] // 64,
                                              np.arange(DI)[:, None]]
                    xprojJ[gl, dr, j, 1] = Wc[2 * j + p[None, :] // 64,
                                              np.arange(DI)[:, None]]
                A = np.exp(ii["A_log"][gg, dr])  # (DI, DS); dt_t is -dt
                for j in range(NJ):
                    A_col[gl, dr, :, j] = A[p % 64, 2 * j + p // 64]
                dsk[gl, dr, :, 0] = np.tile(ii["Dskip"][gg, dr], 2)
        # pack to the exact SBUF layouts (single DMA each)
        conv4Tp = np.zeros((128, 16 * 128), np.float32)
        dtWTp = np.zeros((DI, 4 * 128), np.float32)
        xprojJp = np.zeros((DI, 4 * NJ * 2 * 128), np.float32)
        acolp = np.zeros((128, 4 * NJ), np.float32)
        cb4 = np.zeros((128, 4), np.float32)
        db4 = np.zeros((128, 4), np.float32)
        dk4 = np.zeros((128, 4), np.float32)
        for gl in range(2):
            for dr in range(2):
                i4 = gl * 2 + dr
                for k in range(DC):
                    blk = conv4T[gl, dr, k]          # (DI, 128)
                    conv4Tp[0:64, (i4 * 4 + k) * 128:(i4 * 4 + k + 1) * 128] = blk
                    conv4Tp[64:128, (i4 * 4 + k) * 128:(i4 * 4 + k + 1) * 128] = blk
                dtWTp[:, i4 * 128:(i4 + 1) * 128] = dtWT[gl, dr]
                for j in range(NJ):
                    for sd_ in range(2):
                        o = ((i4 * NJ + j) * 2 + sd_) * 128
                        xprojJp[:, o:o + 128] = xprojJ[gl, dr, j, sd_]
                acolp[:, i4 * NJ:(i4 + 1) * NJ] = A_col[gl, dr]
                cb4[:, i4] = convb[gl, dr, :, 0]
                db4[:, i4] = dtb[gl, dr, :, 0]
                dk4[:, i4] = dsk[gl, dr, :, 0]
        w.update(winTu=_bf(winTu), winTz=_bf(winTz), zb=zb,
                 ub_neg=_bf(np.tile(-ub[:, None], (1, 3))),
                 conv4T=_bf(conv4Tp), convb=cb4, dtWT=_bf(dtWTp), dtb=db4,
                 xprojJ=_bf(xprojJp), A_col=acolp, dsk=dk4, woutT=_bf(woutT))
        maps_w.append(w)

    in_maps = []
    for k in range(NCORE):
        b, gs = k // 2, k % 2
        m = dict(maps_w[gs])
        xp = np.zeros((C, 66, 66), np.float32)
        xp[:, 1:65, 1:65] = x[b]
        m["xpad"] = _bf(np.ascontiguousarray(xp.reshape(C, 66 * 66)))
        in_maps.append(m)
    return in_maps


_CACHE = {}


def kernel(**inputs):
    from concourse.bass_utils import run_bass_kernel_spmd
    if "nc" not in _CACHE:
        _CACHE["nc"] = _build_nc()
    nc = _CACHE["nc"]
    in_maps = _prep_inputs(inputs)
    res = run_bass_kernel_spmd(nc, in_maps, list(range(NCORE))).results
    out = np.stack([np.asarray(res[2 * b]["outp"]).reshape(OUT, H, W)
                    for b in range(B)])
    return out.astype(np.float32)


# revision 50
# speedup vs baseline: 1.4956x; 1.0139x over previous
"""Trainium2 Bass kernel for CDMamba ModifiedSRCMLayer (self-contained).

Sharding: 8 cores; core k handles batch k//2 and mamba group-pair k%2
(groups {0,1} or {2,3}). Group outputs are exchanged with a paired
AllGather; the post-stage (gate blend + output projection) is computed
redundantly on both cores of a pair and the host reads even cores.

v2: all-bf16 datapath. Phase A/C in bf16 packed matmuls, grouped into
activation-table passes (Sqrt | Sigmoid | Silu). Phase B front-end uses
AF.Silu / AF.Softplus directly; the j-loop broadcasts B/C with PE
matmuls, copies PSUM->SBUF bf16 on the ACT engine, and runs dBu/prod as
[128,2048] pure-bf16 DVE tensor_tensor ops (16-bit 2x rate) plus the
DVE tensor_tensor_scan. The s-reduction accumulates in PSUM via matmul.
"""
import sys
import numpy as np

for _p in ("/opt/trn_rl_repo",):
    if _p not in sys.path:
        sys.path.append(_p)

import concourse.bass as bass
import concourse.mybir as mybir
from concourse.bacc import Bacc
from concourse.tile import TileContext

# Model dims (hardcoded per the problem spec)
B, C, H, W = 4, 128, 64, 64
L = H * W                      # 4096
G, DM = 4, 32
DI, DS, DC = 64, 16, 4
DTR = 2
OUT = 128
EPS = 1e-5

NCORE = 8
LC = 512
NCH = L // LC                  # 8
LH = L // 2                    # 2048
NCC = LH // LC                 # 4
NJ = DS // 2                   # 8 j-tiles (2 s-values per tile)
F32 = mybir.dt.float32
BF = mybir.dt.bfloat16
AF = mybir.ActivationFunctionType
ALU = mybir.AluOpType


def _build_nc():
    nc = Bacc(num_devices=NCORE)

    def inp(name, shape, dt=BF):
        return nc.dram_tensor(name, list(shape), dt, kind="ExternalInput")

    xpad = inp("xpad", (C, 66 * 66))
    pe_b = inp("pe_b", (C, L))
    w9 = inp("w9", (C, 9 * 128))
    mred1 = inp("mred1", (128, 1))
    onesr = inp("onesr", (1, 128))
    gateWT = inp("gateWT", (128, 128))
    gateb = inp("gateb", (128, 1), F32)
    winTu = inp("winTu", (C, 128))       # u for both local groups
    winTz = inp("winTz", (C, 128))
    zb = inp("zb", (128, 1), F32)        # ln-affine fold: Win_z @ ln_b
    ub_neg = inp("ub_neg", (128, 3))     # -Win_u @ ln_b (conv halo)
    # pre-packed in SBUF layout (single DMA each)
    conv4T = inp("conv4T", (128, 16 * 128))
    convb = inp("convb", (128, 4), F32)
    dtWT = inp("dtWT", (DI, 4 * 128))
    dtb = inp("dtb", (128, 4), F32)
    xprojJ = inp("xprojJ", (DI, 4 * NJ * 2 * 128))  # replicated B/C weights
    A_col = inp("A_col", (128, 4 * NJ), F32)
    dsk = inp("dsk", (128, 4), F32)
    mredM = inp("mredM", (128, DI))
    woutT = inp("woutT", (128, 2 * DM))
    projT = inp("projT", (128, 128))
    projb = inp("projb", (128, 1), F32)

    ym_loc = nc.dram_tensor("ym_loc", [2 * DM, L], BF)
    ym_all = nc.dram_tensor("ym_all", [C, L], BF)
    outp = nc.dram_tensor("outp", [OUT, L], F32, kind="ExternalOutput")

    with TileContext(nc) as tc:
        with (
            tc.tile_pool(name="const", bufs=1) as cp,
            tc.tile_pool(name="big", bufs=1) as bp,
            tc.tile_pool(name="hpool", bufs=2) as hp,
            tc.tile_pool(name="psP", bufs=1, space="PSUM") as psP,
        ):
            # ---- constants to SBUF ----
            def c_load(ap_dram, shape, nm, dt=BF):
                t = cp.tile(list(shape), dt, name=nm, tag=nm)
                nc.sync.dma_start(t[:], ap_dram)
                return t

            # w9/xpad first (first conv depends on them), split across queues
            w9_sb = cp.tile([C, 9 * 128], BF, name="w9sb", tag="w9sb")
            for qq in range(4):
                eng = (nc.sync, nc.scalar, nc.gpsimd, nc.scalar)[qq]
                eng.dma_start(w9_sb[:, qq * 288:(qq + 1) * 288],
                              w9[:, qq * 288:(qq + 1) * 288])
            mred1_sb = c_load(mred1[:], (128, 1), "mred1sb")
            onesr_sb = c_load(onesr[:], (1, 128), "onesrsb")
            gateWT_sb = c_load(gateWT[:], (128, 128), "gateWTsb")
            gateb_sb = c_load(gateb[:], (128, 1), "gatebsb", F32)
            winTu_sb = c_load(winTu[:], (C, 128), "winTusb")
            winTz_sb = c_load(winTz[:], (C, 128), "winTzsb")
            zb_sb = c_load(zb[:], (128, 1), "zbsb", F32)
            ubn_sb = c_load(ub_neg[:], (128, 3), "ubnsb")
            mredM_sb = c_load(mredM[:], (128, DI), "mredMsb")
            woutT_sb = c_load(woutT[:], (128, 2 * DM), "woutTsb")
            projT_sb = c_load(projT[:], (128, 128), "projTsb")
            projb_sb = c_load(projb[:], (128, 1), "projbsb", F32)

            conv4T_sb = cp.tile([128, 16 * 128], BF)
            nc.scalar.dma_start(conv4T_sb[:], conv4T[:])
            dtWT_sb = cp.tile([DI, 4 * 128], BF)
            nc.gpsimd.dma_start(dtWT_sb[:], dtWT[:])
            xprojJ_sb = cp.tile([DI, 4 * NJ * 2 * 128], BF)
            for qq in range(2):
                eng = (nc.scalar, nc.gpsimd)[qq]
                eng.dma_start(xprojJ_sb[:, qq * 4096:(qq + 1) * 4096],
                              xprojJ[:, qq * 4096:(qq + 1) * 4096])
            acol_sb = cp.tile([128, 4 * NJ], F32)
            nc.sync.dma_start(acol_sb[:], A_col[:])
            convb_sb = cp.tile([128, 4], F32)
            nc.sync.dma_start(convb_sb[:], convb[:])
            dtb_sb = cp.tile([128, 4], F32)
            nc.sync.dma_start(dtb_sb[:], dtb[:])
            dsk_sb = cp.tile([128, 4], F32)
            nc.sync.dma_start(dsk_sb[:], dsk[:])
            eps_sb = cp.tile([1, 1], F32)
            nc.vector.memset(eps_sb[:], EPS)

            # ---- persistent tiles ----
            xs = bp.tile([C, L], BF)
            gate = bp.tile([C, L], BF)
            u_pad = bp.tile([C, L + 6], BF)
            zs = bp.tile([C, L], BF)
            yfb = bp.tile([C, L], BF)

            # halo = -Win_u@ln_b so the folded-LN conv matches zero-padded ref
            nc.vector.tensor_copy(u_pad[:, 0:3], ubn_sb[:])
            nc.vector.tensor_copy(u_pad[:, L + 3:L + 6], ubn_sb[:])

            # ---- Phase A ----
            with tc.tile_pool(name="pA", bufs=2) as pA:
                xpad_sb = pA.tile([C, 66 * 66], BF, bufs=1)
                for qq in range(4):
                    eng = (nc.sync, nc.scalar, nc.gpsimd, nc.scalar)[qq]
                    eng.dma_start(xpad_sb[:, qq * 1089:(qq + 1) * 1089],
                                  xpad[:, qq * 1089:(qq + 1) * 1089])
                xpad3 = xpad_sb[:].rearrange("p (r q) -> p r q", q=66)
                xnc = pA.tile([C, L], BF, bufs=1)   # centered/normed (LN affine folded)
                xcf = pA.tile([C, L], BF, bufs=1)   # centered
                # pass0: pos-enc conv, dense PE burst (no tables)
                for c in range(NCH):
                    cs = slice(c * LC, (c + 1) * LC)
                    pa = psP.tile([128, 8, 64], F32, tag="gen", bufs=2)
                    for tap in range(9):
                        dy, dx = tap // 3, tap % 3
                        nc.tensor.matmul(
                            pa[:],
                            w9_sb[:, tap * 128:(tap + 1) * 128],
                            xpad3[:, c * 8 + dy:c * 8 + dy + 8, dx:dx + 64],
                            start=(tap == 0), stop=(tap == 8))
                    paf = pa[:].rearrange("p a b -> p (a b)")
                    pe_t = pA.tile([128, LC], BF, tag="pe", bufs=3)
                    eng = (nc.scalar, nc.sync, nc.gpsimd)[c % 3]
                    eng.dma_start(pe_t[:], pe_b[:, cs])
                    nc.vector.tensor_tensor(xs[:, cs], paf, pe_t[:], op=ALU.add)
                # pass1: LN in 1024-wide stages (tables: Sqrt; Square free)
                for c2 in range(NCH // 2):
                    c2s = slice(c2 * 2 * LC, (c2 + 1) * 2 * LC)
                    mu = psP.tile([1, 2 * LC], F32, tag="gen", bufs=2)
                    for half in range(2):
                        nc.tensor.matmul(
                            mu[:, half * LC:(half + 1) * LC], mred1_sb[:],
                            xs[:, (c2 * 2 + half) * LC:(c2 * 2 + half + 1) * LC],
                            start=True, stop=True)
                    mu_sb = pA.tile([1, 2 * LC], BF, tag="musb")
                    nc.scalar.copy(mu_sb[:], mu[:])
                    mub = psP.tile([128, 2 * LC], F32, tag="gen", bufs=2)
                    for half in range(2):
                        nc.tensor.matmul(
                            mub[:, half * LC:(half + 1) * LC], onesr_sb[:],
                            mu_sb[:, half * LC:(half + 1) * LC],
                            start=True, stop=True)
                    nc.vector.tensor_tensor(xcf[:, c2s], xs[:, c2s], mub[:],
                                            op=ALU.subtract)
                    xsq = pA.tile([128, 2 * LC], BF, tag="xsq")
                    nc.vector.tensor_tensor(xsq[:], xcf[:, c2s], xcf[:, c2s],
                                            op=ALU.mult)
                    var = psP.tile([1, 2 * LC], F32, tag="gen", bufs=2)
                    for half in range(2):
                        nc.tensor.matmul(
                            var[:, half * LC:(half + 1) * LC], mred1_sb[:],
                            xsq[:, half * LC:(half + 1) * LC],
                            start=True, stop=True)
                    sd = pA.tile([1, 2 * LC], F32, tag="sd")
                    nc.scalar.activation(sd[:], var[:], AF.Sqrt,
                                         bias=eps_sb[:, 0:1])
                    rstd = pA.tile([1, 2 * LC], BF, tag="rstd")
                    with nc.allow_low_precision(reason="bf16 rstd; tol 2e-2"):
                        nc.vector.reciprocal(rstd[:], sd[:])
                    rstdb = psP.tile([128, 2 * LC], F32, tag="gen", bufs=2)
                    for half in range(2):
                        nc.tensor.matmul(
                            rstdb[:, half * LC:(half + 1) * LC], onesr_sb[:],
                            rstd[:, half * LC:(half + 1) * LC],
                            start=True, stop=True)
                    nc.vector.tensor_tensor(xnc[:, c2s], xcf[:, c2s], rstdb[:],
                                            op=ALU.mult)
                # pass2: gate (Sigmoid)
                for c in range(NCH):
                    cs = slice(c * LC, (c + 1) * LC)
                    gps = psP.tile([128, LC], F32, tag="gen", bufs=2)
                    nc.tensor.matmul(gps[:], gateWT_sb[:], xnc[:, cs],
                                     start=True, stop=True)
                    nc.scalar.activation(gate[:, cs], gps[:], AF.Sigmoid,
                                         bias=gateb_sb[:, 0:1])
                # pass3: xz both local groups packed (Silu; Copy free)
                for c in range(NCH):
                    cs = slice(c * LC, (c + 1) * LC)
                    up = psP.tile([128, LC], F32, tag="gen", bufs=2)
                    nc.tensor.matmul(up[:], winTu_sb[:], xnc[:, cs],
                                     start=True, stop=True)
                    nc.scalar.copy(u_pad[:, 3 + c * LC:3 + (c + 1) * LC], up[:])
                    zp = psP.tile([128, LC], F32, tag="gen", bufs=2)
                    nc.tensor.matmul(zp[:], winTz_sb[:], xnc[:, cs],
                                     start=True, stop=True)
                    nc.scalar.activation(zs[:, cs], zp[:], AF.Silu,
                                         bias=zb_sb[:, 0:1])

            # ---- Phase B ----
            # All four (gl,dr) front-ends first (one table set per pass),
            # then the four j-loops (Exp table once).
            with tc.tile_pool(name="pB", bufs=2) as wp:
                uc_all = [bp.tile([128, L], BF, name=f"uc{i4}")
                          for i4 in range(4)]
                dt_all = [bp.tile([128, L], BF, name=f"dtt{i4}")
                          for i4 in range(4)]
                # FE pass1: conv + silu (Silu table)
                for gl in range(2):
                    rows = slice(gl * 64, gl * 64 + 64)
                    for dr in range(2):
                        i4 = gl * 2 + dr
                        uc = uc_all[i4]
                        for cc in range(NCH // 2):
                            ucp = psP.tile([128, 2 * LC], F32, tag="gen", bufs=2)
                            for half in range(2):
                                c = cc * 2 + half
                                for k in range(DC):
                                    off = (c * LC + k) if dr == 0 else (3 + c * LC + k)
                                    nc.tensor.matmul(
                                        ucp[:, half * LC:(half + 1) * LC],
                                        conv4T_sb[rows,
                                                  (i4 * 4 + k) * 128:
                                                  (i4 * 4 + k + 1) * 128],
                                        u_pad[rows, off:off + LC],
                                        start=(k == 0), stop=(k == DC - 1))
                            nc.scalar.activation(
                                uc[:, cc * 2 * LC:(cc + 1) * 2 * LC], ucp[:],
                                AF.Silu, bias=convb_sb[:, i4:i4 + 1])
                # FE pass2: dt matmul + sigmoid (Sigmoid table)
                for i4 in range(4):
                    uc, dt_t = uc_all[i4], dt_all[i4]
                    for cc in range(NCH // 2):
                        c2s = slice(cc * 2 * LC, (cc + 1) * 2 * LC)
                        dtp = psP.tile([128, 2 * LC], F32, tag="gen", bufs=2)
                        for half in range(2):
                            c = cc * 2 + half
                            nc.tensor.matmul(
                                dtp[:, half * LC:(half + 1) * LC],
                                dtWT_sb[:, i4 * 128:(i4 + 1) * 128],
                                uc[0:DI, c * LC:(c + 1) * LC],
                                start=True, stop=True)
                        nc.scalar.activation(dt_t[:, c2s], dtp[:],
                                             AF.Sigmoid,
                                             bias=dtb_sb[:, i4:i4 + 1],
                                             scale=-1.0)
                # FE pass3: dt_t = ln(sigmoid(..)) = -softplus (Ln table)
                for i4 in range(4):
                    nc.scalar.activation(dt_all[i4][:], dt_all[i4][:], AF.Ln)

                # j-loops (Exp table; Copy free)
                for gl in range(2):
                    rows = slice(gl * 64, gl * 64 + 64)
                    for dr in range(2):
                        i4 = gl * 2 + dr
                        uc, dt_t = uc_all[i4], dt_all[i4]
                        dtuc = wp.tile([128, L], BF, tag="dtuc", bufs=2)
                        nc.vector.tensor_tensor(dtuc[:], dt_t[:], uc[:],
                                                op=ALU.mult)
                        horder = (0, 1) if dr == 0 else (1, 0)
                        h_prev = [None] * NJ
                        for hf in horder:
                            hs = slice(hf * LH, (hf + 1) * LH)
                            first = (hf == horder[0])
                            ys = [psP.tile([128, LC], F32, tag=f"ys{q}",
                                           bufs=1, name=f"ys{q}")
                                  for q in range(NCC)]
                            prev_prod = None

                            def bcast(side, j):
                                # B/C broadcast for s-pair j, direct from uc
                                bb = wp.tile([128, LH], BF, bufs=3,
                                             tag=("bbB" if side == 0 else "bbC"))
                                wsl = ((i4 * NJ + j) * 2 + side) * 128
                                for p2 in range(2):
                                    bps = psP.tile([128, 2 * LC], F32,
                                                   tag="gen", bufs=2)
                                    for half in range(2):
                                        q = p2 * 2 + half
                                        nc.tensor.matmul(
                                            bps[:, half * LC:(half + 1) * LC],
                                            xprojJ_sb[:, wsl:wsl + 128],
                                            uc[0:DI, hf * LH + q * LC:
                                               hf * LH + (q + 1) * LC],
                                            start=True, stop=True)
                                    nc.scalar.copy(
                                        bb[:, p2 * 2 * LC:(p2 + 1) * 2 * LC],
                                        bps[:])
                                return bb

                            for j in range(NJ):
                                dA = wp.tile([128, LH], BF, tag="dA", bufs=2)
                                nc.scalar.activation(
                                    dA[:], dt_t[:, hs], AF.Exp,
                                    scale=acol_sb[:, i4 * NJ + j:i4 * NJ + j + 1])
                                bbB = bcast(0, j)
                                dBu = wp.tile([128, LH], BF, tag="dBu")
                                nc.vector.tensor_tensor(dBu[:], dtuc[:, hs],
                                                        bbB[:], op=ALU.mult)
                                h = hp.tile([128, LH], BF, tag="h")
                                hc = hp.tile([128, 1], BF, tag=f"hc{j}",
                                             name=f"hc{j}")
                                init = 0.0 if first else h_prev[j][:, 0:1]
                                if dr == 0:
                                    nc.vector.tensor_tensor_scan(
                                        h[:], dA[:], dBu[:], init,
                                        op0=ALU.mult, op1=ALU.add)
                                    nc.gpsimd.tensor_copy(hc[:], h[:, LH - 1:LH])
                                else:
                                    nc.vector.tensor_tensor_scan(
                                        h[:, ::-1], dA[:, ::-1], dBu[:, ::-1],
                                        init, op0=ALU.mult, op1=ALU.add)
                                    nc.gpsimd.tensor_copy(hc[:], h[:, 0:1])
                                h_prev[j] = hc
                                bbC = bcast(1, j)
                                prod = wp.tile([128, LH], BF, tag="prod")
                                nc.vector.tensor_tensor(prod[:], h[:], bbC[:],
                                                        op=ALU.mult)
                                if prev_prod is not None:
                                    for q in range(NCC):
                                        nc.tensor.matmul(
                                            ys[q][rows, :], mredM_sb[:, 0:DI],
                                            prev_prod[:, q * LC:(q + 1) * LC],
                                            start=(j - 1 == 0), stop=False)
                                prev_prod = prod
                            for q in range(NCC):
                                nc.tensor.matmul(
                                    ys[q][rows, :], mredM_sb[:, 0:DI],
                                    prev_prod[:, q * LC:(q + 1) * LC],
                                    start=(NJ == 1), stop=True)
                            # epilogue for this half
                            for q in range(NCC):
                                c = hf * NCC + q
                                cs = slice(c * LC, (c + 1) * LC)
                                y1 = wp.tile([128, LC], BF, tag="y1")
                                nc.vector.scalar_tensor_tensor(
                                    y1[rows, :], uc[rows, cs],
                                    dsk_sb[rows, i4:i4 + 1],
                                    ys[q][rows, :], op0=ALU.mult,
                                    op1=ALU.subtract)
                                if dr == 0:
                                    nc.gpsimd.tensor_tensor(yfb[rows, cs],
                                                            y1[rows, :],
                                                            zs[rows, cs],
                                                            op=ALU.mult)
                                else:
                                    y2 = wp.tile([128, LC], BF, tag="y2")
                                    nc.gpsimd.tensor_tensor(y2[rows, :],
                                                            y1[rows, :],
                                                            zs[rows, cs],
                                                            op=ALU.mult)
                                    nc.gpsimd.tensor_tensor(yfb[rows, cs],
                                                            yfb[rows, cs],
                                                            y2[rows, :],
                                                            op=ALU.add)
            # ---- Phase C: Wout, exchange, blend, proj ----
            with tc.tile_pool(name="pC", bufs=2) as wpc:
                for c in range(NCH):
                    cs = slice(c * LC, (c + 1) * LC)
                    ymp = psP.tile([2 * DM, LC], F32, tag="gen", bufs=2)
                    nc.tensor.matmul(ymp[:], woutT_sb[:], yfb[:, cs],
                                     start=True, stop=True)
                    ym_sb = wpc.tile([2 * DM, LC], BF, tag="ymsb")
                    nc.scalar.copy(ym_sb[:], ymp[:])
                    eng = (nc.sync, nc.scalar, nc.gpsimd, nc.sync)[c % 4]
                    eng.dma_start(ym_loc[:, cs], ym_sb[:])
                nc.gpsimd.collective_compute(
                    "AllGather", ALU.bypass,
                    replica_groups=[[0, 1], [2, 3], [4, 5], [6, 7]],
                    ins=[ym_loc[:]], outs=[ym_all[:]])
                for c in range(NCH):
                    cs = slice(c * LC, (c + 1) * LC)
                    xm_t = wpc.tile([C, LC], BF, tag="xmt", bufs=3)
                    eng = (nc.sync, nc.scalar, nc.gpsimd, nc.sync)[c % 4]
                    eng.dma_start(xm_t[:], ym_all[:, cs])
                    ta = wpc.tile([128, LC], BF, tag="ta")
                    nc.vector.tensor_tensor(ta[:], xm_t[:], xs[:, cs],
                                            op=ALU.subtract)
                    tb2 = wpc.tile([128, LC], BF, tag="tb")
                    nc.vector.tensor_tensor(tb2[:], gate[:, cs], ta[:],
                                            op=ALU.mult)
                    tc2 = wpc.tile([128, LC], BF, tag="tc")
                    nc.vector.tensor_tensor(tc2[:], xs[:, cs], tb2[:],
                                            op=ALU.add)
                    op_ = psP.tile([128, LC], F32, tag="gen", bufs=2)
                    nc.tensor.matmul(op_[:], projT_sb[:], tc2[:],
                                     start=True, stop=True)
                    osb = wpc.tile([128, LC], F32, tag="osb", bufs=3)
                    nc.scalar.activation(osb[:], op_[:], AF.Identity,
                                         bias=projb_sb[:, 0:1])
                    eng = (nc.sync, nc.gpsimd, nc.scalar, nc.sync)[c % 4]
                    eng.dma_start(outp[:, cs], osb[:])
    nc.finalize()
    return nc


def _bf(a):
    import concourse.mybir as _mb
    return np.asarray(a).astype(_mb.dt.np(_mb.dt.bfloat16))


def _prep_inputs(inputs):
    """Build the 8 per-core in_maps from full inputs."""
    ii = {k: np.asarray(v, dtype=np.float32) for k, v in inputs.items()}
    x = ii["x"]

    maps_w = []  # weight dicts per group-set gs=0,1
    for gs in range(2):
        w = {}
        w9 = np.zeros((C, 9 * 128), np.float32)
        for tap in range(9):
            dy, dx = tap // 3, tap % 3
            blk = np.zeros((C, 128), np.float32)
            np.fill_diagonal(blk, ii["pos_conv_w"][:, 0, dy, dx])
            if tap == 4:
                blk[np.arange(C), np.arange(C)] += 1.0
            w9[:, tap * 128:(tap + 1) * 128] = blk
        w["w9"] = _bf(w9)
        w["pe_b"] = _bf(np.ascontiguousarray(ii["pos_embed"][0].T)
                        + ii["pos_conv_b"][:, None])
        w["mred1"] = _bf(np.full((128, 1), 1.0 / 128, np.float32))
        w["onesr"] = _bf(np.ones((1, 128), np.float32))
        lng = ii["ln_g"]
        lnb = ii["ln_b"]
        # LN affine folded into consumers: xnc on-device is (x-mu)/sd
        w["gateWT"] = _bf(ii["gate_W"].T * lng[:, None])
        w["gateb"] = np.ascontiguousarray(
            (ii["gate_b"] + ii["gate_W"] @ lnb)[:, None])
        w["projT"] = _bf(ii["proj_W"].T)
        w["projb"] = np.ascontiguousarray(ii["proj_b"][:, None])
        w["mredM"] = _bf(np.tile(np.eye(DI, dtype=np.float32), (2, 1)))
        winTu = np.zeros((C, 128), np.float32)
        winTz = np.zeros((C, 128), np.float32)
        zb = np.zeros((128, 1), np.float32)
        ub = np.zeros((128,), np.float32)
        conv4T = np.zeros((2, 2, DC, DI, 128), np.float32)
        convb = np.zeros((2, 2, 128, 1), np.float32)
        dtWT = np.zeros((2, 2, DI, 128), np.float32)
        dtb = np.zeros((2, 2, 128, 1), np.float32)
        xprojJ = np.zeros((2, 2, NJ, 2, DI, 128), np.float32)
        A_col = np.zeros((2, 2, 128, NJ), np.float32)
        dsk = np.zeros((2, 2, 128, 1), np.float32)
        woutT = np.zeros((128, 2 * DM), np.float32)
        for gl in range(2):
            gg = gs * 2 + gl
            gsl = slice(gg * DM, (gg + 1) * DM)
            Wu = ii["m_Win"][gg, 0:DI, :]        # (DI, DM)
            Wz = ii["m_Win"][gg, DI:2 * DI, :]
            winTu[gsl, gl * DI:(gl + 1) * DI] = (Wu * lng[None, gsl]).T
            winTz[gsl, gl * DI:(gl + 1) * DI] = (Wz * lng[None, gsl]).T
            ub[gl * DI:(gl + 1) * DI] = Wu @ lnb[gsl]
            zb[gl * DI:(gl + 1) * DI, 0] = Wz @ lnb[gsl]
            woutT[gl * 64:(gl + 1) * 64, gl * DM:(gl + 1) * DM] = ii["m_Wout"][gg].T
            for dr in range(2):
                for k in range(DC):
                    wk = ii["conv_w"][gg, dr, :, k if dr == 0 else DC - 1 - k]
                    blk = np.zeros((DI, 128), np.float32)
                    blk[np.arange(DI), np.arange(DI)] = wk
                    blk[np.arange(DI), 64 + np.arange(DI)] = wk
                    conv4T[gl, dr, k] = blk
                convb[gl, dr, :, 0] = np.tile(
                    ii["conv_b"][gg, dr]
                    + ii["conv_w"][gg, dr].sum(-1) * ub[gl * DI:(gl + 1) * DI], 2)
                M2 = ii["dt_W"][gg, dr] @ ii["xproj_W"][gg, dr][0:DTR, :]
                dtWT[gl, dr] = np.concatenate([M2.T, M2.T], axis=1)
                dtb[gl, dr, :, 0] = -np.tile(ii["dt_b"][gg, dr], 2)
                Wb = ii["xproj_W"][gg, dr][DTR:DTR + DS, :]        # (DS, DI)
                Wc = ii["xproj_W"][gg, dr][DTR + DS:DTR + 2 * DS, :]
                p = np.arange(128)
                for j in range(NJ):
                    xprojJ[gl, dr, j, 0] = Wb[2 * j + p[None, :] // 64,
                                              np.arange(DI)[:, None]]
                    xprojJ[gl, dr, j, 1] = Wc[2 * j + p[None, :] // 64,
                                              np.arange(DI)[:, None]]
                A = np.exp(ii["A_log"][gg, dr])  # (DI, DS); dt_t is -dt
                for j in range(NJ):
                    A_col[gl, dr, :, j] = A[p % 64, 2 * j + p // 64]
                dsk[gl, dr, :, 0] = np.tile(ii["Dskip"][gg, dr], 2)
        # pack to the exact SBUF layouts (single DMA each)
        conv4Tp = np.zeros((128, 16 * 128), np.float32)
        dtWTp = np.zeros((DI, 4 * 128), np.float32)
        xprojJp = np.zeros((DI, 4 * NJ * 2 * 128), np.float32)
        acolp = np.zeros((128, 4 * NJ), np.float32)
        cb4 = np.zeros((128, 4), np.float32)
        db4 = np.zeros((128, 4), np.float32)
        dk4 = np.zeros((128, 4), np.float32)
        for gl in range(2):
            for dr in range(2):
                i4 = gl * 2 + dr
                for k in range(DC):
                    blk = conv4T[gl, dr, k]          # (DI, 128)
                    conv4Tp[0:64, (i4 * 4 + k) * 128:(i4 * 4 + k + 1) * 128] = blk
                    conv4Tp[64:128, (i4 * 4 + k) * 128:(i4 * 4 + k + 1) * 128] = blk
                dtWTp[:, i4 * 128:(i4 + 1) * 128] = dtWT[gl, dr]
                for j in range(NJ):
                    for sd_ in range(2):
                        o = ((i4 * NJ + j) * 2 + sd_) * 128
                        xprojJp[:, o:o + 128] = xprojJ[gl, dr, j, sd_]
                acolp[:, i4 * NJ:(i4 + 1) * NJ] = A_col[gl, dr]
                cb4[:, i4] = convb[gl, dr, :, 0]
                db4[:, i4] = dtb[gl, dr, :, 0]
                dk4[:, i4] = dsk[gl, dr, :, 0]
        w.update(winTu=_bf(winTu), winTz=_bf(winTz), zb=zb,
                 ub_neg=_bf(np.tile(-ub[:, None], (1, 3))),
                 conv4T=_bf(conv4Tp), convb=cb4, dtWT=_bf(dtWTp), dtb=db4,
                 xprojJ=_bf(xprojJp), A_col=acolp, dsk=dk4, woutT=_bf(woutT))
        maps_w.append(w)

    in_maps = []
    for k in range(NCORE):
        b, gs = k // 2, k % 2
        m = dict(maps_w[gs])
        xp = np.zeros((C, 66, 66), np.float32)
        xp[:, 1:65, 1:65] = x[b]
        m["xpad"] = _bf(np.ascontiguousarray(xp.reshape(C, 66 * 66)))
        in_maps.append(m)
    return in_maps


_CACHE = {}


def kernel(**inputs):
    from concourse.bass_utils import run_bass_kernel_spmd
    if "nc" not in _CACHE:
        _CACHE["nc"] = _build_nc()
    nc = _CACHE["nc"]
    in_maps = _prep_inputs(inputs)
    res = run_bass_kernel_spmd(nc, in_maps, list(range(NCORE))).results
    out = np.stack([np.asarray(res[2 * b]["outp"]).reshape(OUT, H, W)
                    for b in range(B)])
    return out.astype(np.float32)
